# revision 1
# baseline (speedup 1.0000x reference)
"""Trainium2 Bass kernel for nn_Attention_30562987278646.

Sharding: 8 cores = 4 batches x 2 head-groups (4 heads each).
Per core: LN(q/k/v slice) -> project with W_in columns of its heads ->
score matrices (cosine + covariance + margin-variance) -> out = S @ f_v
-> partial @ W_out rows. Host sums the 2 head-group partials per batch.

Exact identities used:
 - LN: ln_g folded into W_in on host (W' = diag(g) W); ln_b @ W_in added
   via K=1 accumulating matmuls on device. Device applies (x - mu) * r only.
 - cov term: qc . kc = dots - d*mq*mk -> rank-1 outer product folded as
   extra contraction rows (K=66 matmul: 64 f-rows + means row + ones row).
 - var term: GAMMA=1 and cosine <= 1 mathematically, so
   relu(1 - cos) == 1 - cos; mean_m(1 - cos_nm) = 1 - colsum(cos_nm)/N,
   and colsum(cos_nm)[n] = (sum_m fk_n[:,m]) . fq_n[:,n] -- one tiny matmul.
 - cos_nm == cosine_sim (norms >> 1e-12), computed once.

Everything runs in d-major (transposed) layout so score matrices come out
transposed (S^T[m,n]) and feed the out-stage matmul directly.
"""

import sys
import numpy as np

for _p in ("/opt/trn_rl_repo", "/root/.axon_site/_ro/trn_rl_repo"):
    if _p not in sys.path:
        sys.path.append(_p)

HEADS = 8
DIM_HEAD = 64
LN_EPS = 1e-5
B, N, DIM = 4, 1024, 512
HG = 2                      # head groups (shards along heads)
HPG = HEADS // HG           # heads per group = 4
IG = HPG * DIM_HEAD         # inner dim per group = 256
NT = N // 128               # 8 n-tiles
NC = N // 512               # 2 n-chunks
CC = DIM // 128             # 4 c-chunks


def _build_nc(cos_w: float, cov_w: float, var_w: float):
    import concourse.bass as bass
    import concourse.bacc as bacc
    import concourse.tile as tile
    from concourse import mybir

    f32 = mybir.dt.float32
    f32r = mybir.dt.float32r
    AF = mybir.ActivationFunctionType
    AX = mybir.AxisListType

    def r(ap):
        return ap.bitcast(f32r)

    nc = bacc.Bacc(target_bir_lowering=False, debug=False)
    _lp = nc.allow_low_precision(reason="f32r is 4-byte storage, not low precision")
    _lp.__enter__()

    xin_d = {
        "xq": nc.declare_dram_parameter("xq", [N, DIM], f32, isOutput=False),
        "xk": nc.declare_dram_parameter("xk", [N, DIM], f32, isOutput=False),
        "xv": nc.declare_dram_parameter("xv", [N, DIM], f32, isOutput=False),
    }
    wf = nc.declare_dram_parameter("wf", [DIM, IG], f32, isOutput=False)
    bw = nc.declare_dram_parameter("bw", [64, IG], f32, isOutput=False)
    wo = nc.declare_dram_parameter("wo", [IG, DIM], f32, isOutput=False)
    ident = nc.declare_dram_parameter("ident", [128, 128], f32, isOutput=False)
    sel = nc.declare_dram_parameter("sel", [128, 2], f32, isOutput=False)
    e1 = nc.declare_dram_parameter("e1", [64, 512], f32, isOutput=False)
    eb = nc.declare_dram_parameter("eb", [128, 128], f32, isOutput=False)
    out = nc.declare_dram_parameter("out", [N, DIM], f32, isOutput=True)

    with tile.TileContext(nc) as tc, \
         tc.tile_pool(name="persist", bufs=1) as P, \
         tc.tile_pool(name="stt", bufs=4) as STP, \
         tc.tile_pool(name="small", bufs=6) as SM, \
         tc.tile_pool(name="osb", bufs=8) as OSB, \
         tc.tile_pool(name="psu", bufs=4, space="PSUM") as PSU, \
         tc.tile_pool(name="psc", bufs=2, space="PSUM") as PSC, \
         tc.tile_pool(name="pst", bufs=2, space="PSUM") as PT:

        # ---- constants / weights in SBUF ----
        id_stage = P.tile([128, 128], f32, tag="id_stage", name="id_stage")
        nc.gpsimd.dma_start(out=id_stage, in_=ident[:, :])
        id_sb = P.tile([128, 128], f32, tag="id", name="id_sb")
        nc.scalar.activation(id_sb, id_stage, AF.Copy)
        sel_sb = P.tile([128, 2], f32r, tag="sel", name="sel_sb")
        nc.gpsimd.dma_start(out=sel_sb, in_=sel[:, :].bitcast(f32r))
        e1_sb = P.tile([64, 512], f32r, tag="e1", name="e1_sb")
        nc.gpsimd.dma_start(out=e1_sb, in_=e1[:, :].bitcast(f32r))
        eb_sb = P.tile([128, 128], f32r, tag="eb", name="eb_sb")
        nc.gpsimd.dma_start(out=eb_sb, in_=eb[:, :].bitcast(f32r))
        bw_sb = P.tile([64, IG], f32r, tag="bw", name="bw_sb")
        nc.gpsimd.dma_start(out=bw_sb, in_=bw[:, :].bitcast(f32r))
        eps_sb = P.tile([128, 1], f32, tag="eps", name="eps_sb")
        nc.vector.memset(eps_sb, LN_EPS)
        vw_sb = P.tile([1, 1], f32, tag="vw", name="vw_sb")
        nc.vector.memset(vw_sb, var_w)
        wf_sb = [P.tile([128, IG], f32r, tag=f"wf{c}", name=f"wf{c}") for c in range(CC)]
        for c in range(CC):
            nc.gpsimd.dma_start(out=wf_sb[c], in_=wf[c * 128:(c + 1) * 128, :].bitcast(f32r))
        wo_sb = [P.tile([64, DIM], f32r, tag=f"wo{j}", name=f"wo{j}") for j in range(4)]
        for j in range(4):
            nc.gpsimd.dma_start(out=wo_sb[j], in_=wo[j * 64:(j + 1) * 64, :].bitcast(f32r))

        # ---- persistent activations (projection outputs) ----
        fTq = [P.tile([128, N], f32r, tag=f"fTq{hp}", name=f"fTq{hp}") for hp in range(2)]
        fTk = [P.tile([128, N], f32r, tag=f"fTk{hp}", name=f"fTk{hp}") for hp in range(2)]
        fv_sb = [P.tile([128, IG], f32r, tag=f"fv{mt}", name=f"fv{mt}") for mt in range(NT)]

        # ======== stages A+B under a scoped pool for the xT tiles ========
        with tc.tile_pool(name="xtp", bufs=1) as XT, \
             tc.tile_pool(name="xin", bufs=4) as XIN, \
             tc.tile_pool(name="xdma", bufs=24) as XD:
            xT = {t: [XT.tile([128, N], f32r, tag=f"xT{t}{c}", name=f"xT{t}{c}")
                      for c in range(CC)] for t in ("xq", "xk", "xv")}

            # stage A: load, LN, transpose to c-major
            for t in ("xq", "xk", "xv"):
                for nt in range(NT):
                    xt = XD.tile([128, DIM], f32, tag="xt")
                    nc.gpsimd.dma_start(
                        out=xt, in_=xin_d[t][nt * 128:(nt + 1) * 128, :])
                    stats = SM.tile([128, nc.vector.BN_STATS_DIM], f32,
                                    tag="bns")
                    nc.vector.bn_stats(out=stats, in_=xt)
                    mv = SM.tile([128, nc.vector.BN_AGGR_DIM], f32, tag="bna")
                    nc.vector.bn_aggr(out=mv, in_=stats)
                    std = SM.tile([128, 1], f32, tag="std")
                    nc.scalar.activation(std, mv[:, 1:2], AF.Sqrt, bias=eps_sb)
                    rin = SM.tile([128, 1], f32, tag="rin")
                    nc.vector.reciprocal(rin, std)
                    nmr = SM.tile([128, 1], f32, tag="nmr")
                    nc.vector.tensor_mul(nmr, mv[:, 0:1], rin)
                    nc.vector.tensor_scalar_mul(nmr, nmr, -1.0)
                    zt = XIN.tile([128, DIM], f32, tag="zt")
                    nc.vector.tensor_scalar_mul(zt, xt, rin)
                    xln = XIN.tile([128, DIM], f32, tag="xln")
                    nc.scalar.activation(xln, zt, AF.Identity, bias=nmr)
                    for c in range(CC):
                        pt = PT.tile([128, 128], f32, tag="pt")
                        nc.tensor.transpose(
                            pt, xln[:, c * 128:(c + 1) * 128], id_sb)
                        nc.scalar.activation(
                            xT[t][c][:, nt * 128:(nt + 1) * 128], pt,
                            AF.Copy)

            # stage B: projections (fp32r)
            for tname, fT in (("xq", fTq), ("xk", fTk)):
                for hp in range(2):
                    for ncx in range(NC):
                        pf = PSU.tile([128, 512], f32, tag="big")
                        for c in range(CC):
                            nc.tensor.matmul(
                                pf,
                                r(wf_sb[c][:, hp * 128:(hp + 1) * 128]),
                                r(xT[tname][c][:, ncx * 512:(ncx + 1) * 512]),
                                start=(c == 0), stop=False)
                        nc.tensor.matmul(
                            pf, r(bw_sb[:, hp * 128:(hp + 1) * 128]),
                            r(e1_sb[0:64, 0:512]), start=False, stop=True)
                        nc.vector.tensor_copy(
                            fT[hp][:, ncx * 512:(ncx + 1) * 512], pf)
            for mt in range(NT):
                pf = PSU.tile([128, IG], f32, tag="big")
                for c in range(CC):
                    nc.tensor.matmul(
                        pf, r(xT["xv"][c][:, mt * 128:(mt + 1) * 128]),
                        r(wf_sb[c]), start=(c == 0), stop=False)
                nc.tensor.matmul(pf, r(e1_sb[0:64, 0:128]), r(bw_sb),
                                 start=False, stop=True)
                nc.vector.tensor_copy(fv_sb[mt], pf)

        # ---- stages C-E under a second persist pool (xT memory now free) ----
        with tc.tile_pool(name="p2", bufs=1) as P2:
            fqn = [P2.tile([128, N], f32r, tag=f"fqn{hp}", name=f"fqn{hp}")
                   for hp in range(2)]
            fkn = [P2.tile([128, N], f32r, tag=f"fkn{hp}", name=f"fkn{hp}")
                   for hp in range(2)]
            fqc = [P2.tile([128, N], f32r, tag=f"fqc{hp}", name=f"fqc{hp}")
                   for hp in range(2)]
            # per-head [1,N] stat rows packed at 32-aligned partition bases.
            # Matmul pairs need EQUAL bases on both operands, so each quantity
            # gets its own tile with heads 0-2 at rows 0/32/64, head 3 at row 0
            # of a sibling tile. ONESP provides an all-ones row at each base.
            RP = [P2.tile([97, N], f32r, tag=f"RP{q}", name=f"RP{q}")
                  for q in range(3)]
            RPB = [P2.tile([33, N], f32r, tag=f"RPB{q}", name=f"RPB{q}")
                   for q in range(3)]
            ONESP = P2.tile([97, 128], f32r, tag="ONESP", name="ONESP")
            zst = P2.tile([128, N], f32, tag="zst", name="zst")
            nc.vector.memset(zst, 0.0)
            for q in range(3):
                nc.scalar.activation(RP[q], zst[0:97, :], AF.Copy)
                nc.scalar.activation(RPB[q], zst[0:33, :], AF.Copy)
            ost = P2.tile([97, 128], f32, tag="ost", name="ost")
            nc.vector.memset(ost, 0.0)
            for b in (0, 32, 64):
                nc.vector.memset(ost[b:b + 1, :], 1.0)
            nc.scalar.activation(ONESP, ost, AF.Copy)

            def row(q, h):
                if h < 3:
                    return RP[q][32 * h:32 * h + 1, :]
                return RPB[q][0:1, :]

            def blk(q, h):
                if h < 3:
                    return RP[q][32 * h:32 * h + 32, :]
                return RPB[q][0:32, :]

            def ones_blk(h):
                if h < 3:
                    return ONESP[32 * h:32 * h + 32, 0:128]
                return e1_sb[0:32, 0:128]

            MK, NMQ, VR = 0, 1, 2
            fks = [P2.tile([128, 1], f32r, tag=f"fks{hp}", name=f"fks{hp}")
                   for hp in range(2)]
            oTh = [P2.tile([64, N], f32r, tag=f"oTh{h}", name=f"oTh{h}")
                   for h in range(HPG)]

            # ======== stage C: stats, norms ========
            with tc.tile_pool(name="rows", bufs=1) as RW:
                qsr = [RW.tile([128, N], f32r, tag=f"qsr{hp}", name=f"qsr{hp}")
                       for hp in range(2)]
                ksr = [RW.tile([128, N], f32r, tag=f"ksr{hp}", name=f"ksr{hp}")
                       for hp in range(2)]

                for t_ in qsr + ksr:
                    nc.scalar.activation(t_, zst, AF.Copy)

                def srow(tiles, h):
                    return tiles[h // 2][(h % 2) * 64:(h % 2) * 64 + 1, :]
                # per-head column sums of f and f^2 via M=1 selector matmuls
                for fT, dsq, dsm in ((fTq, qsr, NMQ), (fTk, ksr, MK)):
                    for hp in range(2):
                        sq = STP.tile([128, N], f32r, tag="sq")
                        nc.scalar.activation(sq, fT[hp], AF.Square)
                        for hj in range(2):
                            h = 2 * hp + hj
                            for ncx in range(NC):
                                cs = slice(ncx * 512, (ncx + 1) * 512)
                                p1 = PSU.tile([1, 512], f32, tag="big")
                                nc.tensor.matmul(p1, r(sel_sb[:, hj:hj + 1]),
                                                 r(fT[hp][:, cs]),
                                                 start=True, stop=True)
                                nc.vector.tensor_copy(row(dsm, h)[:, cs], p1)
                                p2 = PSU.tile([1, 512], f32, tag="big")
                                nc.tensor.matmul(p2, r(sel_sb[:, hj:hj + 1]),
                                                 r(sq[:, cs]),
                                                 start=True, stop=True)
                                nc.vector.tensor_copy(srow(dsq, h)[:, cs], p2)
                for h in range(HPG):
                    # qsr: sum(q^2)->cos_w/qn ; ksr: sum(k^2)->1/kn (in place)
                    qr, kr = srow(qsr, h), srow(ksr, h)
                    nc.scalar.activation(qr, qr, AF.Sqrt)
                    nc.vector.reciprocal(qr, qr)
                    nc.vector.tensor_scalar_mul(qr, qr, cos_w)
                    nc.scalar.activation(kr, kr, AF.Sqrt)
                    nc.vector.reciprocal(kr, kr)
                    nc.vector.tensor_scalar_mul(row(MK, h), row(MK, h),
                                                1.0 / DIM_HEAD)
                    nc.vector.tensor_scalar_mul(row(NMQ, h), row(NMQ, h),
                                                -cov_w / DIM_HEAD)
                # broadcast per-head rows across 64 partitions -> fqn/fkn
                for hp in range(2):
                    for ncx in range(NC):
                        cs = slice(ncx * 512, (ncx + 1) * 512)
                        pb = PSU.tile([128, 512], f32, tag="big")
                        nc.tensor.matmul(pb, r(eb_sb),
                                         r(qsr[hp][:, cs]),
                                         start=True, stop=True)
                        nc.vector.tensor_mul(fqn[hp][:, cs],
                                             fTq[hp][:, cs], pb)
                        pb2 = PSU.tile([128, 512], f32, tag="big")
                        nc.tensor.matmul(pb2, r(eb_sb),
                                         r(ksr[hp][:, cs]),
                                         start=True, stop=True)
                        nc.vector.tensor_mul(fkn[hp][:, cs],
                                             fTk[hp][:, cs], pb2)
                    nc.vector.tensor_scalar_mul(fqc[hp], fTq[hp],
                                                cov_w / DIM_HEAD)
                    nc.vector.reduce_sum(fks[hp], fkn[hp], axis=AX.X)
            # var rows: vr = var_w * (1 - colsum(cos)/N)
            for h in range(HPG):
                hp, ds = h // 2, (h % 2) * 64
                for ncx in range(NC):
                    cs = slice(ncx * 512, (ncx + 1) * 512)
                    pv = PSU.tile([1, 512], f32, tag="big")
                    nc.tensor.matmul(
                        pv, r(fks[hp][ds:ds + 64, 0:1]),
                        r(fqn[hp][ds:ds + 64, cs]),
                        start=True, stop=True)
                    nc.scalar.activation(
                        row(VR, h)[:, cs], pv, AF.Identity,
                        bias=vw_sb, scale=-(var_w / (N * cos_w)))

            # ======== stage D: scores + out-stage ========
            di = 0
            for ncx in range(NC):
                cs = slice(ncx * 512, (ncx + 1) * 512)
                for hp in range(2):
                    for hj in range(2):
                        h = 2 * hp + hj
                        ds = (h % 2) * 64
                        po = PSU.tile([64, 512], f32, tag="big")
                        for mt in range(NT):
                            ms = slice(mt * 128, (mt + 1) * 128)
                            pss = PSC.tile([128, 512], f32, tag="pss")
                            nc.tensor.matmul(
                                pss, r(fkn[hp][ds:ds + 64, ms]),
                                r(fqn[hp][ds:ds + 64, cs]),
                                start=True, stop=False)
                            nc.tensor.matmul(
                                pss, r(fTk[hp][ds:ds + 64, ms]),
                                r(fqc[hp][ds:ds + 64, cs]),
                                start=False, stop=False)
                            nc.tensor.matmul(
                                pss, r(blk(MK, h)[:, ms]),
                                r(blk(NMQ, h)[:, cs]),
                                start=False, stop=False)
                            nc.tensor.matmul(
                                pss, r(ones_blk(h)),
                                r(blk(VR, h)[:, cs]),
                                start=False, stop=True)
                            st = STP.tile([128, 512], f32r, tag="st")
                            if di % 2 == 0:
                                nc.vector.tensor_copy(st, pss)
                            else:
                                nc.scalar.activation(st, pss, AF.Copy)
                            di += 1
                            nc.tensor.matmul(
                                po,
                                r(fv_sb[mt][:, h * 64:(h + 1) * 64]),
                                r(st), start=(mt == 0), stop=(mt == NT - 1))
                        nc.scalar.activation(
                            oTh[h][:, ncx * 512:(ncx + 1) * 512], po, AF.Copy)

            # ======== stage E: W_out projection + store ========
            for nt in range(NT):
                pf = PSU.tile([128, 512], f32, tag="big")
                for j in range(4):
                    nc.tensor.matmul(
                        pf, r(oTh[j][:, nt * 128:(nt + 1) * 128]),
                        r(wo_sb[j]), start=(j == 0), stop=(j == 3))
                ob = OSB.tile([128, 512], f32, tag="ob")
                nc.vector.tensor_copy(ob, pf)
                nc.gpsimd.dma_start(out=out[nt * 128:(nt + 1) * 128, :],
                                    in_=ob)

    _lp.__exit__(None, None, None)
    nc.compile()
    return nc


def _prep(q, k, v, ln_g, ln_b, W_in, W_out, b_out, cov_w_raw, var_w_raw):
    q = np.asarray(q, np.float32)
    k = np.asarray(k, np.float32)
    v = np.asarray(v, np.float32)
    ln_g = np.asarray(ln_g, np.float32)
    ln_b = np.asarray(ln_b, np.float32)
    W_in = np.asarray(W_in, np.float32)
    W_out = np.asarray(W_out, np.float32)

    cov_w = float(1.0 / (1.0 + np.exp(-np.float64(cov_w_raw))))
    var_w = float(1.0 / (1.0 + np.exp(-np.float64(var_w_raw))))
    cos_w = 1.0 - cov_w - var_w

    nc = _build_nc(cos_w, cov_w, var_w)

    W_f = (ln_g[:, None] * W_in).astype(np.float32)      # [512, 512]
    bW = (ln_b @ W_in).astype(np.float32)                # [512]
    ident = np.eye(128, dtype=np.float32)
    sel = np.zeros((128, 2), np.float32)
    sel[:64, 0] = 1.0
    sel[64:, 1] = 1.0
    e1 = np.zeros((64, 512), np.float32)
    e1[0, :] = 1.0
    eb = np.zeros((128, 128), np.float32)
    eb[0, :64] = 1.0
    eb[64, 64:] = 1.0

    in_maps = []
    for core in range(8):
        b, g = core // HG, core % HG
        in_maps.append({
            "xq": np.ascontiguousarray(q[b]),
            "xk": np.ascontiguousarray(k[b]),
            "xv": np.ascontiguousarray(v[b]),
            "wf": np.ascontiguousarray(W_f[:, g * IG:(g + 1) * IG]),
            "bw": np.ascontiguousarray(
                np.concatenate([bW[None, g * IG:(g + 1) * IG],
                                np.zeros((63, IG), np.float32)], axis=0)),
            "wo": np.ascontiguousarray(W_out[g * IG:(g + 1) * IG, :]),
            "ident": ident, "sel": sel, "e1": e1, "eb": eb,
        })
    return nc, in_maps


def kernel(q, k, v, ln_g, ln_b, W_in, W_out, b_out, cov_w_raw, var_w_raw):
    from concourse.bass_utils import run_bass_kernel_spmd

    b_out = np.asarray(b_out, np.float32)
    nc, in_maps = _prep(q, k, v, ln_g, ln_b, W_in, W_out, b_out,
                        cov_w_raw, var_w_raw)
    res = run_bass_kernel_spmd(nc, in_maps, list(range(8)))
    parts = [res.results[c]["out"] for c in range(8)]
    out = np.stack([parts[2 * b] + parts[2 * b + 1] + b_out
                    for b in range(B)])
    return out.astype(np.float32)



# revision 14
# speedup vs baseline: 1.3617x; 1.3617x over previous
"""Trainium2 Bass kernel for nn_Attention_30562987278646.

Sharding: 8 cores = 4 batches x 2 head-groups (4 heads each).
Per core: LN(q/k/v) -> project -> score matrices -> out = S @ f_v ->
partial @ W_out rows. Host sums the 2 head-group partials per batch.

Key identities / layout:
 - LN applied as one fused activation (x*r + (-mu*r)) per n-tile; ln_g
   folded into W on host.
 - cov term: qc . kc == fq . kc (centering q is free), and kc = C f_k
   with C = I - 1/64 folded into the k-side weights on host. So the
   score matmul is ONE K=128 matmul per (m-tile, n-chunk):
     rows 0:64   kc_h      x  fqc_h (= cov_w/64 * fq)
     rows 64:128 fkn_h     x  fqn_h (= cos_w * fq / qn)
 - var term: relu(1-cos)=1-cos (GAMMA=1, |cos|<=1), mean_m(1-cos) gives
   a per-n row vr; its contribution to the output is rank-1
   (vr[n] * colsum_m(f_v)[e]) and is added in the out-stage PSUM
   accumulation as one K=1 matmul per (head, n-chunk).
"""

import sys
import numpy as np

for _p in ("/opt/trn_rl_repo", "/root/.axon_site/_ro/trn_rl_repo"):
    if _p not in sys.path:
        sys.path.append(_p)

HEADS = 8
DIM_HEAD = 64
LN_EPS = 1e-5
B, N, DIM = 4, 1024, 512
HG = 2                      # head groups (shards along heads)
HPG = HEADS // HG           # heads per group = 4
IG = HPG * DIM_HEAD         # inner dim per group = 256
NT = N // 128               # 8 n-tiles
NC = N // 512               # 2 n-chunks
CC = DIM // 128             # 4 c-chunks


def _build_nc(cos_w: float, cov_w: float, var_w: float, has_bias: bool):
    import concourse.bass as bass
    import concourse.bacc as bacc
    import concourse.tile as tile
    from concourse import mybir

    f32 = mybir.dt.float32
    f32r = mybir.dt.float32r
    AF = mybir.ActivationFunctionType
    ALU = mybir.AluOpType
    AX = mybir.AxisListType

    def r(ap):
        return ap.bitcast(f32r)

    nc = bacc.Bacc(target_bir_lowering=False, debug=False)
    _lp = nc.allow_low_precision(reason="f32r is 4-byte storage, not low precision")
    _lp.__enter__()

    xin_d = {
        t: nc.declare_dram_parameter(t, [128, NT * DIM], f32, isOutput=False)
        for t in ("xk", "xq", "xv")
    }
    wk_d = nc.declare_dram_parameter("wk", [128, HPG * DIM], f32, isOutput=False)
    cst_d = nc.declare_dram_parameter("cst", [128, 769], f32, isOutput=False)
    wv_d = nc.declare_dram_parameter("wv", [128, CC * IG], f32, isOutput=False)
    wo_d = nc.declare_dram_parameter("wo", [128, 2 * DIM], f32, isOutput=False)
    ident_d = nc.declare_dram_parameter("ident", [128, 128], f32, isOutput=False)
    if has_bias:
        bq_d = nc.declare_dram_parameter("bq", [1, IG], f32, isOutput=False)
        bk_d = nc.declare_dram_parameter("bk", [1, HPG * 128], f32, isOutput=False)
    out_d = nc.declare_dram_parameter("out", [128, NT * DIM], f32, isOutput=True)

    # engine rotation for bulk copies: scalar (Act) / vector (DVE) / gpsimd
    rot = {"i": 0}

    def copy_rr(dst, src, seq=(0, 1)):
        e = seq[rot["i"] % len(seq)]
        rot["i"] += 1
        if e == 0:
            nc.scalar.activation(dst, src, AF.Copy)
        elif e == 1:
            nc.vector.tensor_copy(dst, src)
        else:
            nc.gpsimd.tensor_copy(dst, src)

    with tile.TileContext(nc) as tc, \
         tc.tile_pool(name="persist", bufs=1) as P:

        # ---- persistent constants ----
        ident_sb = P.tile([128, 128], f32r, name="ident_sb")
        nc.gpsimd.dma_start(out=ident_sb, in_=ident_d[:, :].bitcast(f32r))
        cst = P.tile([128, 769], f32r, name="cst")
        nc.gpsimd.dma_start(out=cst, in_=cst_d[:, :].bitcast(f32r))
        browq = cst[:, 0:128]
        browk = cst[:, 128:256]
        onescol = cst[:, 256:257]
        ones_row = cst[0:1, 257:769]
        eps_sb = P.tile([128, 1], f32, name="eps_sb")
        nc.vector.memset(eps_sb, LN_EPS)
        vwcol = P.tile([97, 1], f32, name="vwcol")
        nc.vector.memset(vwcol, var_w)

        # ---- persistent weights ----
        wv_sb = P.tile([128, CC * IG], f32r, name="wv_sb")
        nc.gpsimd.dma_start(out=wv_sb, in_=wv_d[:, :].bitcast(f32r))
        wo_sb = P.tile([128, 2 * DIM], f32r, name="wo_sb")
        nc.gpsimd.dma_start(out=wo_sb, in_=wo_d[:, :].bitcast(f32r))
        if has_bias:
            bq_sb = P.tile([1, IG], f32r, name="bq_sb")
            nc.gpsimd.dma_start(out=bq_sb, in_=bq_d[:, :].bitcast(f32r))
            bk_sb = P.tile([1, HPG * 128], f32r, name="bk_sb")
            nc.gpsimd.dma_start(out=bk_sb, in_=bk_d[:, :].bitcast(f32r))

        # ---- persistent activations ----
        # L[h]: rows 0:64 kc_h, rows 64:128 fk_h -> fkn_h   [128, N]
        # R[h]: rows 0:64 fqc_h, rows 64:128 fq_h -> fqn_h  [128, N]
        L = [P.tile([128, N], f32r, name=f"L{h}") for h in range(HPG)]
        R = [P.tile([128, N], f32r, name=f"R{h}") for h in range(HPG)]
        fv = [P.tile([128, IG], f32r, name=f"fv{mt}") for mt in range(NT)]
        oT = [P.tile([128, N], f32r, name=f"oT{j}") for j in range(2)]
        # stat rows at partition 32h: qstat = cos_w-ready 1/qn, kstat = 1/kn
        qstat = P.tile([97, N], f32r, name="qstat")
        kstat = P.tile([97, N], f32r, name="kstat")
        vrr = P.tile([97, N], f32r, name="vrr")
        fkst = P.tile([128, HPG], f32r, name="fkst")
        frep = P.tile([97, IG], f32r, name="frep")

        zst = P.tile([97, N], f32, name="zst")
        nc.vector.memset(zst, 1.0)
        nc.scalar.activation(qstat, zst, AF.Copy)
        nc.scalar.activation(kstat, zst, AF.Copy)

        # ======== stages A+B: load, LN, transpose, project ========
        with tc.tile_pool(name="xa", bufs=1) as XA, \
             tc.tile_pool(name="zt", bufs=4) as ZT, \
             tc.tile_pool(name="sqp", bufs=4) as SQP, \
             tc.tile_pool(name="smal", bufs=2) as SM, \
             tc.tile_pool(name="wkp", bufs=1) as WKP, \
             tc.tile_pool(name="pt", bufs=2, space="PSUM") as PT, \
             tc.tile_pool(name="pb", bufs=2, space="PSUM") as PB, \
             tc.tile_pool(name="pstat", bufs=2, space="PSUM") as PST:

            wk_sb = WKP.tile([128, HPG * DIM], f32r, name="wk_sb")
            nc.gpsimd.dma_start(out=wk_sb, in_=wk_d[:, :].bitcast(f32r))

            xin = {}
            xT = {}
            for t in ("xk", "xq", "xv"):
                xin[t] = XA.tile([128, NT * DIM], f32, tag=f"xin{t}",
                                 name=f"xin{t}")
                for hf in range(2):
                    cs = slice(hf * (NT * DIM // 2), (hf + 1) * (NT * DIM // 2))
                    nc.gpsimd.dma_start(out=xin[t][:, cs], in_=xin_d[t][:, cs])
                xT[t] = XA.tile([128, CC * N], f32r, tag=f"xT{t}",
                                name=f"xT{t}")

            # --- stage A per tensor: LN stats (batched), fused LN, transpose
            for t in ("xk", "xq", "xv"):
                mvt = SM.tile([128, 2 * NT], f32, tag="mvt", name="mvt")
                stt = SQP.tile([128, nc.vector.BN_STATS_DIM], f32, tag="bns", name="stt")
                for nt in range(NT):
                    nc.vector.bn_stats(
                        out=stt, in_=xin[t][:, nt * DIM:(nt + 1) * DIM])
                    nc.vector.bn_aggr(out=mvt[:, 2 * nt:2 * nt + 2], in_=stt)
                mv3 = mvt.rearrange("p (t s) -> p t s", s=2)
                rin = SM.tile([128, NT], f32, tag="rin", name="rin")
                nc.scalar.activation(rin, mv3[:, :, 1:2], AF.Sqrt, bias=eps_sb)
                nc.vector.reciprocal(rin, rin)
                nmr = SM.tile([128, NT], f32, tag="nmr", name="nmr")
                nc.vector.tensor_tensor(nmr, mv3[:, :, 0:1], rin, ALU.mult)
                nc.vector.tensor_scalar_mul(nmr, nmr, -1.0)
                xT3 = xT[t].rearrange("p (c n) -> p c n", c=CC)
                for nt in range(NT):
                    zt = ZT.tile([128, DIM], f32r, tag="zt", name="zt")
                    if nt % 2 == 0:
                        nc.scalar.activation(
                            zt, xin[t][:, nt * DIM:(nt + 1) * DIM],
                            AF.Identity, bias=nmr[:, nt:nt + 1],
                            scale=rin[:, nt:nt + 1])
                    else:
                        nc.vector.tensor_scalar(
                            zt, xin[t][:, nt * DIM:(nt + 1) * DIM],
                            rin[:, nt:nt + 1], nmr[:, nt:nt + 1],
                            ALU.mult, ALU.add)
                    pt = PT.tile([128, 512], f32, tag="pt", name="pt")
                    for c in range(CC):
                        nc.tensor.transpose(
                            r(pt[:, c * 128:(c + 1) * 128]),
                            r(zt[:, c * 128:(c + 1) * 128]), r(ident_sb))
                    copy_rr(xT3[:, :, nt * 128:(nt + 1) * 128], pt)

            # --- stage B-k: aug projection -> L, stats (kstat rows hold kn)
            for h in range(HPG):
                for ncx in range(NC):
                    cs = slice(ncx * 512, (ncx + 1) * 512)
                    pf = PB.tile([128, 512], f32, tag="pf", name="pf")
                    for c in range(CC):
                        nc.tensor.matmul(
                            pf, r(wk_sb[:, h * DIM + c * 128:h * DIM + (c + 1) * 128]),
                            r(xT["xk"][:, c * N + ncx * 512:c * N + (ncx + 1) * 512]),
                            start=(c == 0), stop=(c == 3 and not has_bias))
                    if has_bias:
                        nc.tensor.matmul(
                            pf, r(bk_sb[0:1, h * 128:(h + 1) * 128]),
                            r(ones_row[0:1, :]), start=False, stop=True)
                    copy_rr(L[h][:, cs], pf)
                    sq = SQP.tile([64, 512], f32r, tag="sq", name="sq")
                    nc.scalar.activation(sq, pf[64:128, :], AF.Square)
                    ps1 = PST.tile([1, 512], f32, tag="ps", name="ps1")
                    nc.tensor.matmul(ps1, r(onescol[0:64, :]), r(sq),
                                     start=True, stop=True)
                    nc.scalar.activation(kstat[32 * h:32 * h + 1, cs], ps1,
                                         AF.Sqrt)
            nc.vector.reciprocal(kstat, kstat)

            # --- stage B-q: plain projection (shared weights wv) -> R, stats
            for hp in range(2):
                for ncx in range(NC):
                    cs = slice(ncx * 512, (ncx + 1) * 512)
                    pf = PB.tile([128, 512], f32, tag="pf", name="pf")
                    for c in range(CC):
                        nc.tensor.matmul(
                            pf, r(wv_sb[:, c * IG + hp * 128:c * IG + (hp + 1) * 128]),
                            r(xT["xq"][:, c * N + ncx * 512:c * N + (ncx + 1) * 512]),
                            start=(c == 0), stop=(c == 3 and not has_bias))
                    if has_bias:
                        nc.tensor.matmul(
                            pf, r(bq_sb[0:1, hp * 128:(hp + 1) * 128]),
                            r(ones_row[0:1, :]), start=False, stop=True)
                    for j in range(2):
                        h = 2 * hp + j
                        fq = pf[j * 64:(j + 1) * 64, :]
                        nc.vector.tensor_scalar_mul(
                            R[h][0:64, cs], fq, cov_w / DIM_HEAD)
                        copy_rr(R[h][64:128, cs], fq, seq=(0, 1))
                        sq = SQP.tile([64, 512], f32r, tag="sq", name="sq")
                        nc.scalar.activation(sq, fq, AF.Square)
                        ps1 = PST.tile([1, 512], f32, tag="ps", name="ps1")
                        nc.tensor.matmul(ps1, r(onescol[0:64, :]), r(sq),
                                         start=True, stop=True)
                        nc.scalar.activation(qstat[32 * h:32 * h + 1, cs],
                                             ps1, AF.Sqrt)
            nc.vector.reciprocal(qstat, qstat)

            # --- stage B-v: projection -> fv (n-major), fvsum
            pfs = PST.tile([1, IG], f32, tag="pfs", name="pfs")
            for mt in range(NT):
                pfv = PB.tile([128, 512], f32, tag="pf", name="pfv")[:, 0:IG]
                for c in range(CC):
                    nc.tensor.matmul(
                        pfv, r(xT["xv"][:, c * N + mt * 128:c * N + (mt + 1) * 128]),
                        r(wv_sb[:, c * IG:(c + 1) * IG]),
                        start=(c == 0), stop=(c == 3 and not has_bias))
                if has_bias:
                    nc.tensor.matmul(
                        pfv, r(ones_row[0:1, 0:128]), r(bq_sb[0:1, :]),
                        start=False, stop=True)
                copy_rr(fv[mt], pfv)
            for mt in range(NT):
                nc.tensor.matmul(pfs, r(onescol[:, :]), r(fv[mt]),
                                 start=(mt == 0), stop=(mt == NT - 1))
            for h in range(HPG):
                nc.scalar.activation(frep[32 * h:32 * h + 1, :], pfs, AF.Copy)

        # ======== stage C: normalize L/R, var rows ========
        with tc.tile_pool(name="pbc", bufs=2, space="PSUM") as PBC, \
             tc.tile_pool(name="pvr", bufs=2, space="PSUM") as PVR:
            for h in range(HPG):
                hs = slice(32 * h, 32 * h + 1)
                for ncx in range(NC):
                    cs = slice(ncx * 512, (ncx + 1) * 512)
                    # k side: fkn = fk * bcast(1/kn); accumulate fks rows
                    pb = PBC.tile([128, 512], f32, tag="pb", name="pb")
                    nc.tensor.matmul(pb, r(browk[hs, :]), r(kstat[hs, cs]),
                                     start=True, stop=True,
                                     tile_position=(32 * h, 0))
                    nc.vector.tensor_tensor(
                        L[h][64:128, cs], L[h][64:128, cs],
                        pb[64:128, :], ALU.mult)
                    # q side: fqn = fq * bcast(cos_w/qn)
                    pb2 = PBC.tile([128, 512], f32, tag="pb", name="pb2")
                    nc.tensor.matmul(pb2, r(browq[hs, :]), r(qstat[hs, cs]),
                                     start=True, stop=True,
                                     tile_position=(32 * h, 0))
                    nc.vector.tensor_tensor(
                        R[h][64:128, cs], R[h][64:128, cs],
                        pb2[64:128, :], ALU.mult)
            for h in range(HPG):
                nc.vector.reduce_sum(fkst[64:128, h:h + 1], L[h][64:128, :],
                                     axis=AX.X)
            # vr rows: vr = var_w - var_w/(N*cos_w) * (fks . fqn)
            for h in range(HPG):
                hs = slice(32 * h, 32 * h + 1)
                for ncx in range(NC):
                    cs = slice(ncx * 512, (ncx + 1) * 512)
                    pv1 = PVR.tile([1, 512], f32, tag="pvr", name="pv1")
                    nc.tensor.matmul(
                        pv1, r(fkst[64:128, h:h + 1]),
                        r(R[h][64:128, cs]), start=True, stop=True)
                    nc.scalar.activation(
                        vrr[hs, cs], pv1, AF.Identity, bias=vwcol[0:1, :],
                        scale=-(var_w / (N * cos_w)))

        # ======== stage D: scores + out-stage ========
        with tc.tile_pool(name="pss", bufs=2, space="PSUM") as PSS, \
             tc.tile_pool(name="pop", bufs=1, space="PSUM") as POP, \
             tc.tile_pool(name="stp", bufs=4) as STP:
            for j2 in range(2):
                po = {(jj, ncx): POP.tile([64, 512], f32, tag=f"po{jj}{ncx}",
                                          name=f"po{j2}_{jj}_{ncx}")
                      for jj in range(2) for ncx in range(NC)}
                for jj in range(2):
                    h = 2 * j2 + jj
                    hs = slice(32 * h, 32 * h + 1)
                    for mt in range(NT):
                        ms = slice(mt * 128, (mt + 1) * 128)
                        pss = PSS.tile([128, 1024], f32, tag="pss", name="pss")
                        nc.tensor.matmul(pss[:, 0:512], r(L[h][:, ms]),
                                         r(R[h][:, 0:512]),
                                         start=True, stop=True)
                        nc.tensor.matmul(pss[:, 512:1024], r(L[h][:, ms]),
                                         r(R[h][:, 512:1024]),
                                         start=True, stop=True)
                        st = STP.tile([128, 1024], f32r, tag="st", name="st")
                        copy_rr(st, pss, seq=(0, 1))
                        for ncx in range(NC):
                            nc.tensor.matmul(
                                po[(jj, ncx)],
                                r(fv[mt][:, h * 64:(h + 1) * 64]),
                                r(st[:, ncx * 512:(ncx + 1) * 512]),
                                start=(mt == 0), stop=False)
                    for ncx in range(NC):
                        cs = slice(ncx * 512, (ncx + 1) * 512)
                        nc.tensor.matmul(
                            po[(jj, ncx)],
                            r(frep[hs, h * 64:(h + 1) * 64]),
                            r(vrr[hs, cs]), start=False, stop=True,
                            tile_position=(32 * h, 0))
                for jj in range(2):
                    for ncx in range(NC):
                        cs = slice(ncx * 512, (ncx + 1) * 512)
                        copy_rr(oT[j2][jj * 64:(jj + 1) * 64, cs],
                                po[(jj, ncx)])

        # ======== stage E: W_out projection + store ========
        with tc.tile_pool(name="pe2", bufs=2, space="PSUM") as PE2, \
             tc.tile_pool(name="obp", bufs=2) as OBP:
            for nt2 in range(NT // 2):
                obt = OBP.tile([128, 2 * DIM], f32, tag="ob", name="obt")
                for j in range(2):
                    nt = 2 * nt2 + j
                    pf = PE2.tile([128, 512], f32, tag="pf", name="pfe")
                    for j2 in range(2):
                        nc.tensor.matmul(
                            pf, r(oT[j2][:, nt * 128:(nt + 1) * 128]),
                            r(wo_sb[:, j2 * 512:(j2 + 1) * 512]),
                            start=(j2 == 0), stop=(j2 == 1))
                    copy_rr(obt[:, j * DIM:(j + 1) * DIM], pf)
                nc.gpsimd.dma_start(
                    out=out_d[:, nt2 * 2 * DIM:(nt2 + 1) * 2 * DIM], in_=obt)

    _lp.__exit__(None, None, None)
    nc.compile()
    return nc


def _host_prep_weights(ln_g, ln_b, W_in, W_out, g):
    """Per-head-group weight layouts (see _build_nc docstring)."""
    W_f = (ln_g[:, None] * W_in)[:, g * IG:(g + 1) * IG]  # [512, 256]
    C = np.eye(DIM_HEAD, dtype=np.float32) - 1.0 / DIM_HEAD

    # k-aug per head: [W_h @ C | W_h] -> [512, 128] each
    wk = np.empty((DIM, HPG * 128), np.float32)
    for h in range(HPG):
        Wh = W_f[:, h * 64:(h + 1) * 64]
        wk[:, h * 128:h * 128 + 64] = Wh @ C
        wk[:, h * 128 + 64:(h + 1) * 128] = Wh
    # c-major SBUF layouts: [p, h*512 + c*128 + i] = wk[c*128+p, h*128+i]
    wk_sb = np.ascontiguousarray(
        wk.reshape(CC, 128, HPG, 128).transpose(1, 2, 0, 3).reshape(128, HPG * DIM))
    wv_sb = np.ascontiguousarray(
        W_f.reshape(CC, 128, IG).transpose(1, 0, 2).reshape(128, CC * IG))
    Wo = W_out[g * IG:(g + 1) * IG, :]  # [256, 512]
    wo_sb = np.ascontiguousarray(
        Wo.reshape(2, 128, DIM).transpose(1, 0, 2).reshape(128, 2 * DIM))

    bW = (ln_b @ W_in)[g * IG:(g + 1) * IG].astype(np.float32)  # [256]
    has_bias = bool(np.any(bW))
    bq = bW[None, :]
    bk = np.empty((1, HPG * 128), np.float32)
    for h in range(HPG):
        bh = bW[h * 64:(h + 1) * 64]
        bk[0, h * 128:h * 128 + 64] = bh @ C
        bk[0, h * 128 + 64:(h + 1) * 128] = bh
    return wk_sb, wv_sb, wo_sb, bq, bk, has_bias


def _prep(q, k, v, ln_g, ln_b, W_in, W_out, b_out, cov_w_raw, var_w_raw):
    q = np.asarray(q, np.float32)
    k = np.asarray(k, np.float32)
    v = np.asarray(v, np.float32)
    ln_g = np.asarray(ln_g, np.float32)
    ln_b = np.asarray(ln_b, np.float32)
    W_in = np.asarray(W_in, np.float32)
    W_out = np.asarray(W_out, np.float32)

    cov_w = float(1.0 / (1.0 + np.exp(-np.float64(cov_w_raw))))
    var_w = float(1.0 / (1.0 + np.exp(-np.float64(var_w_raw))))
    cos_w = 1.0 - cov_w - var_w

    per_g = [_host_prep_weights(ln_g, ln_b, W_in, W_out, g) for g in range(HG)]
    has_bias = any(pg[5] for pg in per_g)
    nc = _build_nc(cos_w, cov_w, var_w, has_bias)

    ident = np.eye(128, dtype=np.float32)
    cst = np.zeros((128, 769), np.float32)
    for h in range(HPG):
        cst[32 * h, 64:128] = cos_w      # browq (mult by cos_w/qn)
        cst[32 * h, 128 + 64:128 + 128] = 1.0  # browk (divide by kn)
    cst[:, 256] = 1.0                    # onescol
    cst[0, 257:769] = 1.0                # ones_row

    def pmaj(x2d):  # [1024, 512] -> [128, 8*512] p-major
        return np.ascontiguousarray(
            x2d.reshape(NT, 128, DIM).transpose(1, 0, 2).reshape(128, NT * DIM))

    in_maps = []
    for core in range(8):
        b, g = core // HG, core % HG
        wk_sb, wv_sb, wo_sb, bq, bk, _ = per_g[g]
        m = {
            "xq": pmaj(q[b]), "xk": pmaj(k[b]), "xv": pmaj(v[b]),
            "wk": wk_sb, "wv": wv_sb, "wo": wo_sb, "ident": ident,
            "cst": cst,
        }
        if has_bias:
            m["bq"] = bq
            m["bk"] = bk
        in_maps.append(m)
    return nc, in_maps


def kernel(q, k, v, ln_g, ln_b, W_in, W_out, b_out, cov_w_raw, var_w_raw):
    from concourse.bass_utils import run_bass_kernel_spmd

    b_out = np.asarray(b_out, np.float32)
    nc, in_maps = _prep(q, k, v, ln_g, ln_b, W_in, W_out, b_out,
                        cov_w_raw, var_w_raw)
    res = run_bass_kernel_spmd(nc, in_maps, list(range(8)))

    def unpmaj(o):  # [128, 8*512] -> [1024, 512]
        return o.reshape(128, NT, DIM).transpose(1, 0, 2).reshape(N, DIM)

    parts = [unpmaj(res.results[c]["out"]) for c in range(8)]
    out = np.stack([parts[2 * b] + parts[2 * b + 1] + b_out
                    for b in range(B)])
    return out.astype(np.float32)


# revision 15
# speedup vs baseline: 1.5146x; 1.1123x over previous
"""Trainium2 Bass kernel for nn_Attention_30562987278646.

Sharding: 8 cores = 4 batches x 2 head-groups (4 heads each).
Per core: LN(q/k/v) -> project -> score matrices -> out = S @ f_v ->
partial @ W_out rows. Host sums the 2 head-group partials per batch.

Key identities / layout:
 - LN applied as one fused activation (x*r + (-mu*r)) per n-tile; ln_g
   folded into W on host.
 - cov term: qc . kc == fq . kc (centering q is free), and kc = C f_k
   with C = I - 1/64 folded into the k-side weights on host. So the
   score matmul is ONE K=128 matmul per (m-tile, n-chunk):
     rows 0:64   kc_h      x  fqc_h (= cov_w/64 * fq)
     rows 64:128 fkn_h     x  fqn_h (= cos_w * fq / qn)
 - var term: relu(1-cos)=1-cos (GAMMA=1, |cos|<=1), mean_m(1-cos) gives
   a per-n row vr; its contribution to the output is rank-1
   (vr[n] * colsum_m(f_v)[e]) and is added in the out-stage PSUM
   accumulation as one K=1 matmul per (head, n-chunk).
"""

import sys
import numpy as np

for _p in ("/opt/trn_rl_repo", "/root/.axon_site/_ro/trn_rl_repo"):
    if _p not in sys.path:
        sys.path.append(_p)

HEADS = 8
DIM_HEAD = 64
LN_EPS = 1e-5
B, N, DIM = 4, 1024, 512
HG = 2                      # head groups (shards along heads)
HPG = HEADS // HG           # heads per group = 4
IG = HPG * DIM_HEAD         # inner dim per group = 256
NT = N // 128               # 8 n-tiles
NC = N // 512               # 2 n-chunks
CC = DIM // 128             # 4 c-chunks


def _build_nc(cos_w: float, cov_w: float, var_w: float, has_bias: bool):
    import concourse.bass as bass
    import concourse.bacc as bacc
    import concourse.tile as tile
    from concourse import mybir

    f32 = mybir.dt.float32
    f32r = mybir.dt.float32r
    AF = mybir.ActivationFunctionType
    ALU = mybir.AluOpType
    AX = mybir.AxisListType

    def r(ap):
        return ap.bitcast(f32r)

    nc = bacc.Bacc(target_bir_lowering=False, debug=False)
    _lp = nc.allow_low_precision(reason="f32r is 4-byte storage, not low precision")
    _lp.__enter__()

    xin_d = {
        t: nc.declare_dram_parameter(t, [128, NT * DIM], f32, isOutput=False)
        for t in ("xk", "xq", "xv")
    }
    wk_d = nc.declare_dram_parameter("wk", [128, HPG * DIM], f32, isOutput=False)
    cst_d = nc.declare_dram_parameter("cst", [128, 769], f32, isOutput=False)
    wv_d = nc.declare_dram_parameter("wv", [128, CC * IG], f32, isOutput=False)
    wo_d = nc.declare_dram_parameter("wo", [128, 2 * DIM], f32, isOutput=False)
    ident_d = nc.declare_dram_parameter("ident", [128, 128], f32, isOutput=False)
    if has_bias:
        bq_d = nc.declare_dram_parameter("bq", [1, IG], f32, isOutput=False)
        bk_d = nc.declare_dram_parameter("bk", [1, HPG * 128], f32, isOutput=False)
    out_d = nc.declare_dram_parameter("out", [128, NT * DIM], f32, isOutput=True)

    # engine rotation for bulk copies: scalar (Act) / vector (DVE) / gpsimd
    rot = {"i": 0}

    def copy_rr(dst, src, seq=(0, 1)):
        e = seq[rot["i"] % len(seq)]
        rot["i"] += 1
        if e == 0:
            nc.scalar.activation(dst, src, AF.Copy)
        elif e == 1:
            nc.vector.tensor_copy(dst, src)
        else:
            nc.gpsimd.tensor_copy(dst, src)

    with tile.TileContext(nc) as tc, \
         tc.tile_pool(name="persist", bufs=1) as P:

        # ---- persistent constants ----
        ident_sb = P.tile([128, 128], f32r, name="ident_sb")
        nc.sync.dma_start(out=ident_sb, in_=ident_d[:, :].bitcast(f32r))
        cst = P.tile([128, 769], f32r, name="cst")
        nc.sync.dma_start(out=cst, in_=cst_d[:, :].bitcast(f32r))
        browq = cst[:, 0:128]
        browk = cst[:, 128:256]
        onescol = cst[:, 256:257]
        ones_row = cst[0:1, 257:769]
        eps_sb = P.tile([128, 1], f32, name="eps_sb")
        nc.vector.memset(eps_sb, LN_EPS)
        vwcol = P.tile([97, 1], f32, name="vwcol")
        nc.vector.memset(vwcol, var_w)

        # ---- persistent weights ----
        wv_sb = P.tile([128, CC * IG], f32r, name="wv_sb")
        nc.sync.dma_start(out=wv_sb, in_=wv_d[:, :].bitcast(f32r))
        wo_sb = P.tile([128, 2 * DIM], f32r, name="wo_sb")
        nc.sync.dma_start(out=wo_sb, in_=wo_d[:, :].bitcast(f32r))
        if has_bias:
            bq_sb = P.tile([1, IG], f32r, name="bq_sb")
            nc.sync.dma_start(out=bq_sb, in_=bq_d[:, :].bitcast(f32r))
            bk_sb = P.tile([1, HPG * 128], f32r, name="bk_sb")
            nc.sync.dma_start(out=bk_sb, in_=bk_d[:, :].bitcast(f32r))

        # ---- persistent activations ----
        # L[h]: rows 0:64 kc_h, rows 64:128 fk_h -> fkn_h   [128, N]
        # R[h]: rows 0:64 fqc_h, rows 64:128 fq_h -> fqn_h  [128, N]
        L = [P.tile([128, N], f32r, name=f"L{h}") for h in range(HPG)]
        R = [P.tile([128, N], f32r, name=f"R{h}") for h in range(HPG)]
        fv = [P.tile([128, IG], f32r, name=f"fv{mt}") for mt in range(NT)]
        oT = [P.tile([128, N], f32r, name=f"oT{j}") for j in range(2)]
        # stat rows at partition 32h: qstat = cos_w-ready 1/qn, kstat = 1/kn
        qstat = P.tile([97, N], f32r, name="qstat")
        kstat = P.tile([97, N], f32r, name="kstat")
        vrr = P.tile([97, N], f32r, name="vrr")
        fkst = P.tile([128, HPG], f32r, name="fkst")
        frep = P.tile([97, IG], f32r, name="frep")

        zst = P.tile([97, N], f32, name="zst")
        nc.vector.memset(zst, 1.0)
        nc.scalar.activation(qstat, zst, AF.Copy)
        nc.scalar.activation(kstat, zst, AF.Copy)

        # ======== stages A+B: load, LN, transpose, project ========
        with tc.tile_pool(name="xa", bufs=1) as XA, \
             tc.tile_pool(name="zt", bufs=4) as ZT, \
             tc.tile_pool(name="sqp", bufs=4) as SQP, \
             tc.tile_pool(name="smal", bufs=2) as SM, \
             tc.tile_pool(name="wkp", bufs=1) as WKP, \
             tc.tile_pool(name="pt", bufs=2, space="PSUM") as PT, \
             tc.tile_pool(name="pb", bufs=2, space="PSUM") as PB, \
             tc.tile_pool(name="pstat", bufs=2, space="PSUM") as PST:

            wk_sb = WKP.tile([128, HPG * DIM], f32r, name="wk_sb")
            nc.sync.dma_start(out=wk_sb, in_=wk_d[:, :].bitcast(f32r))

            xin = {}
            xT = {}
            for t in ("xk", "xq", "xv"):
                xin[t] = XA.tile([128, NT * DIM], f32, tag=f"xin{t}",
                                 name=f"xin{t}")
                for hf in range(2):
                    cs = slice(hf * (NT * DIM // 2), (hf + 1) * (NT * DIM // 2))
                    nc.sync.dma_start(out=xin[t][:, cs], in_=xin_d[t][:, cs])
                xT[t] = XA.tile([128, CC * N], f32r, tag=f"xT{t}",
                                name=f"xT{t}")

            # --- stage A per tensor: LN stats (batched), fused LN, transpose
            for t in ("xk", "xq", "xv"):
                mvt = SM.tile([128, 2 * NT], f32, tag="mvt", name="mvt")
                stt = SQP.tile([128, nc.vector.BN_STATS_DIM], f32, tag="bns", name="stt")
                for nt in range(NT):
                    nc.vector.bn_stats(
                        out=stt, in_=xin[t][:, nt * DIM:(nt + 1) * DIM])
                    nc.vector.bn_aggr(out=mvt[:, 2 * nt:2 * nt + 2], in_=stt)
                mv3 = mvt.rearrange("p (t s) -> p t s", s=2)
                rin = SM.tile([128, NT], f32, tag="rin", name="rin")
                nc.scalar.activation(rin, mv3[:, :, 1:2], AF.Sqrt, bias=eps_sb)
                nc.vector.reciprocal(rin, rin)
                nmr = SM.tile([128, NT], f32, tag="nmr", name="nmr")
                nc.vector.tensor_tensor(nmr, mv3[:, :, 0:1], rin, ALU.mult)
                nc.vector.tensor_scalar_mul(nmr, nmr, -1.0)
                xT3 = xT[t].rearrange("p (c n) -> p c n", c=CC)
                for nt in range(NT):
                    zt = ZT.tile([128, DIM], f32r, tag="zt", name="zt")
                    if nt % 3 == 0:
                        nc.scalar.activation(
                            zt, xin[t][:, nt * DIM:(nt + 1) * DIM],
                            AF.Identity, bias=nmr[:, nt:nt + 1],
                            scale=rin[:, nt:nt + 1])
                    elif nt % 3 == 1:
                        nc.vector.tensor_scalar(
                            zt, xin[t][:, nt * DIM:(nt + 1) * DIM],
                            rin[:, nt:nt + 1], nmr[:, nt:nt + 1],
                            ALU.mult, ALU.add)
                    else:
                        nc.gpsimd.tensor_scalar(
                            zt, xin[t][:, nt * DIM:(nt + 1) * DIM],
                            rin[:, nt:nt + 1], nmr[:, nt:nt + 1],
                            ALU.mult, ALU.add)
                    pt = PT.tile([128, 512], f32, tag="pt", name="pt")
                    for c in range(CC):
                        nc.tensor.transpose(
                            r(pt[:, c * 128:(c + 1) * 128]),
                            r(zt[:, c * 128:(c + 1) * 128]), r(ident_sb))
                    copy_rr(xT3[:, :, nt * 128:(nt + 1) * 128], pt)

            # --- stage B-k: aug projection -> L, stats (kstat rows hold kn)
            for h in range(HPG):
                for ncx in range(NC):
                    cs = slice(ncx * 512, (ncx + 1) * 512)
                    pf = PB.tile([128, 512], f32, tag="pf", name="pf")
                    for c in range(CC):
                        nc.tensor.matmul(
                            pf, r(wk_sb[:, h * DIM + c * 128:h * DIM + (c + 1) * 128]),
                            r(xT["xk"][:, c * N + ncx * 512:c * N + (ncx + 1) * 512]),
                            start=(c == 0), stop=(c == 3 and not has_bias))
                    if has_bias:
                        nc.tensor.matmul(
                            pf, r(bk_sb[0:1, h * 128:(h + 1) * 128]),
                            r(ones_row[0:1, :]), start=False, stop=True)
                    copy_rr(L[h][:, cs], pf)
                    sq = SQP.tile([64, 512], f32r, tag="sq", name="sq")
                    nc.gpsimd.tensor_tensor(sq, L[h][64:128, cs],
                                            L[h][64:128, cs], ALU.mult)
                    ps1 = PST.tile([1, 512], f32, tag="ps", name="ps1")
                    nc.tensor.matmul(ps1, r(onescol[0:64, :]), r(sq),
                                     start=True, stop=True)
                    nc.scalar.activation(kstat[32 * h:32 * h + 1, cs], ps1,
                                         AF.Sqrt)
            nc.vector.reciprocal(kstat, kstat)

            # --- stage B-q: plain projection (shared weights wv) -> R, stats
            for hp in range(2):
                for ncx in range(NC):
                    cs = slice(ncx * 512, (ncx + 1) * 512)
                    pf = PB.tile([128, 512], f32, tag="pf", name="pf")
                    for c in range(CC):
                        nc.tensor.matmul(
                            pf, r(wv_sb[:, c * IG + hp * 128:c * IG + (hp + 1) * 128]),
                            r(xT["xq"][:, c * N + ncx * 512:c * N + (ncx + 1) * 512]),
                            start=(c == 0), stop=(c == 3 and not has_bias))
                    if has_bias:
                        nc.tensor.matmul(
                            pf, r(bq_sb[0:1, hp * 128:(hp + 1) * 128]),
                            r(ones_row[0:1, :]), start=False, stop=True)
                    for j in range(2):
                        h = 2 * hp + j
                        fq = pf[j * 64:(j + 1) * 64, :]
                        if j == 0:
                            nc.vector.tensor_scalar_mul(
                                R[h][0:64, cs], fq, cov_w / DIM_HEAD)
                        else:
                            nc.scalar.mul(R[h][0:64, cs], fq,
                                          cov_w / DIM_HEAD)
                        copy_rr(R[h][64:128, cs], fq, seq=(0, 1))
                        sq = SQP.tile([64, 512], f32r, tag="sq", name="sq")
                        nc.scalar.activation(sq, fq, AF.Square)
                        ps1 = PST.tile([1, 512], f32, tag="ps", name="ps1")
                        nc.tensor.matmul(ps1, r(onescol[0:64, :]), r(sq),
                                         start=True, stop=True)
                        nc.scalar.activation(qstat[32 * h:32 * h + 1, cs],
                                             ps1, AF.Sqrt)
            nc.vector.reciprocal(qstat, qstat)

            # --- stage B-v: projection -> fv (n-major), fvsum
            pfs = PST.tile([1, IG], f32, tag="pfs", name="pfs")
            for mt in range(NT):
                pfv = PB.tile([128, 512], f32, tag="pf", name="pfv")[:, 0:IG]
                for c in range(CC):
                    nc.tensor.matmul(
                        pfv, r(xT["xv"][:, c * N + mt * 128:c * N + (mt + 1) * 128]),
                        r(wv_sb[:, c * IG:(c + 1) * IG]),
                        start=(c == 0), stop=(c == 3 and not has_bias))
                if has_bias:
                    nc.tensor.matmul(
                        pfv, r(ones_row[0:1, 0:128]), r(bq_sb[0:1, :]),
                        start=False, stop=True)
                copy_rr(fv[mt], pfv)
            for mt in range(NT):
                nc.tensor.matmul(pfs, r(onescol[:, :]), r(fv[mt]),
                                 start=(mt == 0), stop=(mt == NT - 1))
            for h in range(HPG):
                nc.scalar.activation(frep[32 * h:32 * h + 1, :], pfs, AF.Copy)

        # ======== stage C: normalize L/R, var rows ========
        with tc.tile_pool(name="pbc", bufs=2, space="PSUM") as PBC, \
             tc.tile_pool(name="pvr", bufs=2, space="PSUM") as PVR:
            for h in range(HPG):
                hs = slice(32 * h, 32 * h + 1)
                for ncx in range(NC):
                    cs = slice(ncx * 512, (ncx + 1) * 512)
                    # k side: fkn = fk * bcast(1/kn); accumulate fks rows
                    pb = PBC.tile([128, 512], f32, tag="pb", name="pb")
                    nc.tensor.matmul(pb, r(browk[hs, :]), r(kstat[hs, cs]),
                                     start=True, stop=True,
                                     tile_position=(32 * h, 0))
                    nc.vector.tensor_tensor(
                        L[h][64:128, cs], L[h][64:128, cs],
                        pb[64:128, :], ALU.mult)
                    # q side: fqn = fq * bcast(cos_w/qn)
                    pb2 = PBC.tile([128, 512], f32, tag="pb", name="pb2")
                    nc.tensor.matmul(pb2, r(browq[hs, :]), r(qstat[hs, cs]),
                                     start=True, stop=True,
                                     tile_position=(32 * h, 0))
                    nc.vector.tensor_tensor(
                        R[h][64:128, cs], R[h][64:128, cs],
                        pb2[64:128, :], ALU.mult)
            for h in range(HPG):
                nc.vector.reduce_sum(fkst[64:128, h:h + 1], L[h][64:128, :],
                                     axis=AX.X)
            # vr rows: vr = var_w - var_w/(N*cos_w) * (fks . fqn)
            for h in range(HPG):
                hs = slice(32 * h, 32 * h + 1)
                for ncx in range(NC):
                    cs = slice(ncx * 512, (ncx + 1) * 512)
                    pv1 = PVR.tile([1, 512], f32, tag="pvr", name="pv1")
                    nc.tensor.matmul(
                        pv1, r(fkst[64:128, h:h + 1]),
                        r(R[h][64:128, cs]), start=True, stop=True)
                    nc.scalar.activation(
                        vrr[hs, cs], pv1, AF.Identity, bias=vwcol[0:1, :],
                        scale=-(var_w / (N * cos_w)))

        # ======== stage D: scores + out-stage ========
        with tc.tile_pool(name="pss", bufs=3, space="PSUM") as PSS, \
             tc.tile_pool(name="pop", bufs=1, space="PSUM") as POP, \
             tc.tile_pool(name="stp", bufs=4) as STP:
            for h in range(HPG):
                j2, jj = h // 2, h % 2
                hs = slice(32 * h, 32 * h + 1)
                po = [POP.tile([64, 512], f32, tag=f"po{ncx}",
                               name=f"po{h}_{ncx}") for ncx in range(NC)]
                for mt in range(NT):
                    ms = slice(mt * 128, (mt + 1) * 128)
                    pss = PSS.tile([128, 1024], f32, tag="pss", name="pss")
                    nc.tensor.matmul(pss[:, 0:512], r(L[h][:, ms]),
                                     r(R[h][:, 0:512]),
                                     start=True, stop=True)
                    nc.tensor.matmul(pss[:, 512:1024], r(L[h][:, ms]),
                                     r(R[h][:, 512:1024]),
                                     start=True, stop=True)
                    st = STP.tile([128, 1024], f32r, tag="st", name="st")
                    copy_rr(st, pss, seq=(0, 1))
                    for ncx in range(NC):
                        nc.tensor.matmul(
                            po[ncx],
                            r(fv[mt][:, h * 64:(h + 1) * 64]),
                            r(st[:, ncx * 512:(ncx + 1) * 512]),
                            start=(mt == 0), stop=False)
                for ncx in range(NC):
                    cs = slice(ncx * 512, (ncx + 1) * 512)
                    nc.tensor.matmul(
                        po[ncx],
                        r(frep[hs, h * 64:(h + 1) * 64]),
                        r(vrr[hs, cs]), start=False, stop=True,
                        tile_position=(32 * h, 0))
                    copy_rr(oT[j2][jj * 64:(jj + 1) * 64, cs], po[ncx])

        # ======== stage E: W_out projection + store ========
        with tc.tile_pool(name="pe2", bufs=2, space="PSUM") as PE2, \
             tc.tile_pool(name="obp", bufs=2) as OBP:
            for nt2 in range(NT // 2):
                obt = OBP.tile([128, 2 * DIM], f32, tag="ob", name="obt")
                for j in range(2):
                    nt = 2 * nt2 + j
                    pf = PE2.tile([128, 512], f32, tag="pf", name="pfe")
                    for j2 in range(2):
                        nc.tensor.matmul(
                            pf, r(oT[j2][:, nt * 128:(nt + 1) * 128]),
                            r(wo_sb[:, j2 * 512:(j2 + 1) * 512]),
                            start=(j2 == 0), stop=(j2 == 1))
                    copy_rr(obt[:, j * DIM:(j + 1) * DIM], pf)
                nc.sync.dma_start(
                    out=out_d[:, nt2 * 2 * DIM:(nt2 + 1) * 2 * DIM], in_=obt)

    _lp.__exit__(None, None, None)
    nc.compile()
    return nc


def _host_prep_weights(ln_g, ln_b, W_in, W_out, g):
    """Per-head-group weight layouts (see _build_nc docstring)."""
    W_f = (ln_g[:, None] * W_in)[:, g * IG:(g + 1) * IG]  # [512, 256]
    C = np.eye(DIM_HEAD, dtype=np.float32) - 1.0 / DIM_HEAD

    # k-aug per head: [W_h @ C | W_h] -> [512, 128] each
    wk = np.empty((DIM, HPG * 128), np.float32)
    for h in range(HPG):
        Wh = W_f[:, h * 64:(h + 1) * 64]
        wk[:, h * 128:h * 128 + 64] = Wh @ C
        wk[:, h * 128 + 64:(h + 1) * 128] = Wh
    # c-major SBUF layouts: [p, h*512 + c*128 + i] = wk[c*128+p, h*128+i]
    wk_sb = np.ascontiguousarray(
        wk.reshape(CC, 128, HPG, 128).transpose(1, 2, 0, 3).reshape(128, HPG * DIM))
    wv_sb = np.ascontiguousarray(
        W_f.reshape(CC, 128, IG).transpose(1, 0, 2).reshape(128, CC * IG))
    Wo = W_out[g * IG:(g + 1) * IG, :]  # [256, 512]
    wo_sb = np.ascontiguousarray(
        Wo.reshape(2, 128, DIM).transpose(1, 0, 2).reshape(128, 2 * DIM))

    bW = (ln_b @ W_in)[g * IG:(g + 1) * IG].astype(np.float32)  # [256]
    has_bias = bool(np.any(bW))
    bq = bW[None, :]
    bk = np.empty((1, HPG * 128), np.float32)
    for h in range(HPG):
        bh = bW[h * 64:(h + 1) * 64]
        bk[0, h * 128:h * 128 + 64] = bh @ C
        bk[0, h * 128 + 64:(h + 1) * 128] = bh
    return wk_sb, wv_sb, wo_sb, bq, bk, has_bias


def _prep(q, k, v, ln_g, ln_b, W_in, W_out, b_out, cov_w_raw, var_w_raw):
    q = np.asarray(q, np.float32)
    k = np.asarray(k, np.float32)
    v = np.asarray(v, np.float32)
    ln_g = np.asarray(ln_g, np.float32)
    ln_b = np.asarray(ln_b, np.float32)
    W_in = np.asarray(W_in, np.float32)
    W_out = np.asarray(W_out, np.float32)

    cov_w = float(1.0 / (1.0 + np.exp(-np.float64(cov_w_raw))))
    var_w = float(1.0 / (1.0 + np.exp(-np.float64(var_w_raw))))
    cos_w = 1.0 - cov_w - var_w

    per_g = [_host_prep_weights(ln_g, ln_b, W_in, W_out, g) for g in range(HG)]
    has_bias = any(pg[5] for pg in per_g)
    nc = _build_nc(cos_w, cov_w, var_w, has_bias)

    ident = np.eye(128, dtype=np.float32)
    cst = np.zeros((128, 769), np.float32)
    for h in range(HPG):
        cst[32 * h, 64:128] = cos_w      # browq (mult by cos_w/qn)
        cst[32 * h, 128 + 64:128 + 128] = 1.0  # browk (divide by kn)
    cst[:, 256] = 1.0                    # onescol
    cst[0, 257:769] = 1.0                # ones_row

    def pmaj(x2d):  # [1024, 512] -> [128, 8*512] p-major
        return np.ascontiguousarray(
            x2d.reshape(NT, 128, DIM).transpose(1, 0, 2).reshape(128, NT * DIM))

    in_maps = []
    for core in range(8):
        b, g = core // HG, core % HG
        wk_sb, wv_sb, wo_sb, bq, bk, _ = per_g[g]
        m = {
            "xq": pmaj(q[b]), "xk": pmaj(k[b]), "xv": pmaj(v[b]),
            "wk": wk_sb, "wv": wv_sb, "wo": wo_sb, "ident": ident,
            "cst": cst,
        }
        if has_bias:
            m["bq"] = bq
            m["bk"] = bk
        in_maps.append(m)
    return nc, in_maps


def kernel(q, k, v, ln_g, ln_b, W_in, W_out, b_out, cov_w_raw, var_w_raw):
    from concourse.bass_utils import run_bass_kernel_spmd

    b_out = np.asarray(b_out, np.float32)
    nc, in_maps = _prep(q, k, v, ln_g, ln_b, W_in, W_out, b_out,
                        cov_w_raw, var_w_raw)
    res = run_bass_kernel_spmd(nc, in_maps, list(range(8)))

    def unpmaj(o):  # [128, 8*512] -> [1024, 512]
        return o.reshape(128, NT, DIM).transpose(1, 0, 2).reshape(N, DIM)

    parts = [unpmaj(res.results[c]["out"]) for c in range(8)]
    out = np.stack([parts[2 * b] + parts[2 * b + 1] + b_out
                    for b in range(B)])
    return out.astype(np.float32)


# revision 16
# speedup vs baseline: 1.6697x; 1.1024x over previous
"""Trainium2 Bass kernel for nn_Attention_30562987278646.

Sharding: 8 cores = 4 batches x 2 head-groups (4 heads each).
Per core: LN(q/k/v) -> project -> score matrices -> out = S @ f_v ->
partial @ W_out rows. Host sums the 2 head-group partials per batch.

Key identities / layout:
 - LN applied as one fused activation (x*r + (-mu*r)) per n-tile; ln_g
   folded into W on host.
 - cov term: qc . kc == fq . kc (centering q is free), and kc = C f_k
   with C = I - 1/64 folded into the k-side weights on host. So the
   score matmul is ONE K=128 matmul per (m-tile, n-chunk):
     rows 0:64   kc_h      x  fqc_h (= cov_w/64 * fq)
     rows 64:128 fkn_h     x  fqn_h (= cos_w * fq / qn)
 - var term: relu(1-cos)=1-cos (GAMMA=1, |cos|<=1), mean_m(1-cos) gives
   a per-n row vr; its contribution to the output is rank-1
   (vr[n] * colsum_m(f_v)[e]) and is added in the out-stage PSUM
   accumulation as one K=1 matmul per (head, n-chunk).
"""

import sys
import numpy as np

for _p in ("/opt/trn_rl_repo", "/root/.axon_site/_ro/trn_rl_repo"):
    if _p not in sys.path:
        sys.path.append(_p)

HEADS = 8
DIM_HEAD = 64
LN_EPS = 1e-5
B, N, DIM = 4, 1024, 512
HG = 2                      # head groups (shards along heads)
HPG = HEADS // HG           # heads per group = 4
IG = HPG * DIM_HEAD         # inner dim per group = 256
NT = N // 128               # 8 n-tiles
NC = N // 512               # 2 n-chunks
CC = DIM // 128             # 4 c-chunks


def _build_nc(cos_w: float, cov_w: float, var_w: float, has_bias: bool):
    import concourse.bass as bass
    import concourse.bacc as bacc
    import concourse.tile as tile
    from concourse import mybir

    f32 = mybir.dt.float32
    f32r = mybir.dt.float32r
    AF = mybir.ActivationFunctionType
    ALU = mybir.AluOpType
    AX = mybir.AxisListType

    def r(ap):
        return ap.bitcast(f32r)

    nc = bacc.Bacc(target_bir_lowering=False, debug=False)
    _lp = nc.allow_low_precision(reason="f32r is 4-byte storage, not low precision")
    _lp.__enter__()

    xin_d = {
        t: nc.declare_dram_parameter(t, [128, NT * DIM], f32, isOutput=False)
        for t in ("xk", "xq", "xv")
    }
    wk_d = nc.declare_dram_parameter("wk", [128, HPG * DIM], f32, isOutput=False)
    cst_d = nc.declare_dram_parameter("cst", [128, 769], f32, isOutput=False)
    wv_d = nc.declare_dram_parameter("wv", [128, CC * IG], f32, isOutput=False)
    wo_d = nc.declare_dram_parameter("wo", [128, 2 * DIM], f32, isOutput=False)
    ident_d = nc.declare_dram_parameter("ident", [128, 128], f32, isOutput=False)
    if has_bias:
        bq_d = nc.declare_dram_parameter("bq", [1, IG], f32, isOutput=False)
        bk_d = nc.declare_dram_parameter("bk", [1, HPG * 128], f32, isOutput=False)
    out_d = nc.declare_dram_parameter("out", [128, NT * DIM], f32, isOutput=True)

    # engine rotation for bulk copies: scalar (Act) / vector (DVE) / gpsimd
    rot = {"i": 0}

    def copy_rr(dst, src, seq=(0, 1)):
        e = seq[rot["i"] % len(seq)]
        rot["i"] += 1
        if e == 0:
            nc.scalar.activation(dst, src, AF.Copy)
        elif e == 1:
            nc.vector.tensor_copy(dst, src)
        else:
            nc.gpsimd.tensor_copy(dst, src)

    with tile.TileContext(nc) as tc, \
         tc.tile_pool(name="persist", bufs=1) as P:

        # ---- persistent constants ----
        ident_sb = P.tile([128, 128], f32r, name="ident_sb")
        nc.sync.dma_start(out=ident_sb, in_=ident_d[:, :].bitcast(f32r))
        cst = P.tile([128, 769], f32r, name="cst")
        nc.sync.dma_start(out=cst, in_=cst_d[:, :].bitcast(f32r))
        browq = cst[:, 0:128]
        browk = cst[:, 128:256]
        onescol = cst[:, 256:257]
        ones_row = cst[0:1, 257:769]
        eps_sb = P.tile([128, 1], f32, name="eps_sb")
        nc.vector.memset(eps_sb, LN_EPS)
        vwcol = P.tile([97, 1], f32, name="vwcol")
        nc.vector.memset(vwcol, var_w)

        # ---- persistent weights ----
        wv_sb = P.tile([128, CC * IG], f32r, name="wv_sb")
        nc.sync.dma_start(out=wv_sb, in_=wv_d[:, :].bitcast(f32r))
        wo_sb = P.tile([128, 2 * DIM], f32r, name="wo_sb")
        nc.sync.dma_start(out=wo_sb, in_=wo_d[:, :].bitcast(f32r))
        if has_bias:
            bq_sb = P.tile([1, IG], f32r, name="bq_sb")
            nc.sync.dma_start(out=bq_sb, in_=bq_d[:, :].bitcast(f32r))
            bk_sb = P.tile([1, HPG * 128], f32r, name="bk_sb")
            nc.sync.dma_start(out=bk_sb, in_=bk_d[:, :].bitcast(f32r))

        # ---- persistent activations ----
        # L[h]: rows 0:64 kc_h, rows 64:128 fk_h -> fkn_h   [128, N]
        # R[h]: rows 0:64 fqc_h, rows 64:128 fq_h -> fqn_h  [128, N]
        L = [P.tile([128, N], f32r, name=f"L{h}") for h in range(HPG)]
        R = [P.tile([128, N], f32r, name=f"R{h}") for h in range(HPG)]
        fv = [P.tile([128, IG], f32r, name=f"fv{mt}") for mt in range(NT)]
        oT = [P.tile([128, N], f32r, name=f"oT{j}") for j in range(2)]
        # stat rows at partition 32h: qstat = cos_w-ready 1/qn, kstat = 1/kn
        qstat = P.tile([97, N], f32r, name="qstat")
        kstat = P.tile([97, N], f32r, name="kstat")
        vrr = P.tile([97, N], f32r, name="vrr")
        fkst = P.tile([128, HPG], f32r, name="fkst")
        frep = P.tile([97, IG], f32r, name="frep")

        zst = P.tile([97, N], f32, name="zst")
        nc.vector.memset(zst, 1.0)
        nc.scalar.activation(qstat, zst, AF.Copy)
        nc.scalar.activation(kstat, zst, AF.Copy)

        # ======== stages A+B: load, LN, transpose, project ========
        with tc.tile_pool(name="xa", bufs=1) as XA, \
             tc.tile_pool(name="zt", bufs=4) as ZT, \
             tc.tile_pool(name="sqp", bufs=4) as SQP, \
             tc.tile_pool(name="smal", bufs=2) as SM, \
             tc.tile_pool(name="wkp", bufs=1) as WKP, \
             tc.tile_pool(name="pt", bufs=2, space="PSUM") as PT, \
             tc.tile_pool(name="pb", bufs=2, space="PSUM") as PB, \
             tc.tile_pool(name="pstat", bufs=2, space="PSUM") as PST:

            wk_sb = WKP.tile([128, HPG * DIM], f32r, name="wk_sb")
            nc.sync.dma_start(out=wk_sb, in_=wk_d[:, :].bitcast(f32r))

            xin = {}
            xT = {}
            for t in ("xk", "xq", "xv"):
                xin[t] = XA.tile([128, NT * DIM], f32, tag=f"xin{t}",
                                 name=f"xin{t}")
                for hf in range(2):
                    cs = slice(hf * (NT * DIM // 2), (hf + 1) * (NT * DIM // 2))
                    nc.sync.dma_start(out=xin[t][:, cs], in_=xin_d[t][:, cs])
                xT[t] = XA.tile([128, CC * N], f32r, tag=f"xT{t}",
                                name=f"xT{t}")

            # --- stage A per tensor: LN stats (batched), fused LN, transpose
            for t in ("xk", "xq", "xv"):
                mvt = SM.tile([128, 2 * NT], f32, tag="mvt", name="mvt")
                stt = SQP.tile([128, nc.vector.BN_STATS_DIM], f32, tag="bns", name="stt")
                for nt in range(NT):
                    nc.vector.bn_stats(
                        out=stt, in_=xin[t][:, nt * DIM:(nt + 1) * DIM])
                    nc.vector.bn_aggr(out=mvt[:, 2 * nt:2 * nt + 2], in_=stt)
                mv3 = mvt.rearrange("p (t s) -> p t s", s=2)
                rin = SM.tile([128, NT], f32, tag="rin", name="rin")
                nc.scalar.activation(rin, mv3[:, :, 1:2], AF.Sqrt, bias=eps_sb)
                nc.vector.reciprocal(rin, rin)
                nmr = SM.tile([128, NT], f32, tag="nmr", name="nmr")
                nc.vector.tensor_tensor(nmr, mv3[:, :, 0:1], rin, ALU.mult)
                nc.vector.tensor_scalar_mul(nmr, nmr, -1.0)
                xT3 = xT[t].rearrange("p (c n) -> p c n", c=CC)
                for nt in range(NT):
                    zt = ZT.tile([128, DIM], f32r, tag="zt", name="zt")
                    nc.gpsimd.tensor_scalar(
                        zt, xin[t][:, nt * DIM:(nt + 1) * DIM],
                        rin[:, nt:nt + 1], nmr[:, nt:nt + 1],
                        ALU.mult, ALU.add)
                    pt = PT.tile([128, 512], f32, tag="pt", name="pt")
                    for c in range(CC):
                        nc.tensor.transpose(
                            r(pt[:, c * 128:(c + 1) * 128]),
                            r(zt[:, c * 128:(c + 1) * 128]), r(ident_sb))
                    copy_rr(xT3[:, :, nt * 128:(nt + 1) * 128], pt)

            # --- stage B-k: aug projection -> L, stats (kstat rows hold kn)
            for h in range(HPG):
                for ncx in range(NC):
                    cs = slice(ncx * 512, (ncx + 1) * 512)
                    pf = PB.tile([128, 512], f32, tag="pf", name="pf")
                    for c in range(CC):
                        nc.tensor.matmul(
                            pf, r(wk_sb[:, h * DIM + c * 128:h * DIM + (c + 1) * 128]),
                            r(xT["xk"][:, c * N + ncx * 512:c * N + (ncx + 1) * 512]),
                            start=(c == 0), stop=(c == 3 and not has_bias))
                    if has_bias:
                        nc.tensor.matmul(
                            pf, r(bk_sb[0:1, h * 128:(h + 1) * 128]),
                            r(ones_row[0:1, :]), start=False, stop=True)
                    copy_rr(L[h][:, cs], pf)
                    sq = SQP.tile([64, 512], f32r, tag="sq", name="sq")
                    nc.gpsimd.tensor_tensor(sq, L[h][64:128, cs],
                                            L[h][64:128, cs], ALU.mult)
                    ps1 = PST.tile([1, 512], f32, tag="ps", name="ps1")
                    nc.tensor.matmul(ps1, r(onescol[0:64, :]), r(sq),
                                     start=True, stop=True)
                    nc.scalar.activation(kstat[32 * h:32 * h + 1, cs], ps1,
                                         AF.Sqrt)
                if h == 1:
                    nc.vector.reciprocal(kstat[0:33, :], kstat[0:33, :])
            nc.vector.reciprocal(kstat[64:97, :], kstat[64:97, :])

            # --- stage B-q: plain projection (shared weights wv) -> R, stats
            for hp in range(2):
                for ncx in range(NC):
                    cs = slice(ncx * 512, (ncx + 1) * 512)
                    pf = PB.tile([128, 512], f32, tag="pf", name="pf")
                    for c in range(CC):
                        nc.tensor.matmul(
                            pf, r(wv_sb[:, c * IG + hp * 128:c * IG + (hp + 1) * 128]),
                            r(xT["xq"][:, c * N + ncx * 512:c * N + (ncx + 1) * 512]),
                            start=(c == 0), stop=(c == 3 and not has_bias))
                    if has_bias:
                        nc.tensor.matmul(
                            pf, r(bq_sb[0:1, hp * 128:(hp + 1) * 128]),
                            r(ones_row[0:1, :]), start=False, stop=True)
                    for j in range(2):
                        h = 2 * hp + j
                        fq = pf[j * 64:(j + 1) * 64, :]
                        if j == 0:
                            nc.vector.tensor_scalar_mul(
                                R[h][0:64, cs], fq, cov_w / DIM_HEAD)
                        else:
                            nc.scalar.mul(R[h][0:64, cs], fq,
                                          cov_w / DIM_HEAD)
                        copy_rr(R[h][64:128, cs], fq, seq=(0, 1))
                        sq = SQP.tile([64, 512], f32r, tag="sq", name="sq")
                        nc.gpsimd.tensor_tensor(sq, R[h][64:128, cs],
                                                R[h][64:128, cs], ALU.mult)
                        ps1 = PST.tile([1, 512], f32, tag="ps", name="ps1")
                        nc.tensor.matmul(ps1, r(onescol[0:64, :]), r(sq),
                                         start=True, stop=True)
                        nc.scalar.activation(qstat[32 * h:32 * h + 1, cs],
                                             ps1, AF.Sqrt)
                if hp == 0:
                    nc.vector.reciprocal(qstat[0:33, :], qstat[0:33, :])
            nc.vector.reciprocal(qstat[64:97, :], qstat[64:97, :])

            # --- stage B-v: projection -> fv (n-major), fvsum
            pfs = PST.tile([1, IG], f32, tag="pfs", name="pfs")
            for mt in range(NT):
                pfv = PB.tile([128, 512], f32, tag="pf", name="pfv")[:, 0:IG]
                for c in range(CC):
                    nc.tensor.matmul(
                        pfv, r(xT["xv"][:, c * N + mt * 128:c * N + (mt + 1) * 128]),
                        r(wv_sb[:, c * IG:(c + 1) * IG]),
                        start=(c == 0), stop=(c == 3 and not has_bias))
                if has_bias:
                    nc.tensor.matmul(
                        pfv, r(ones_row[0:1, 0:128]), r(bq_sb[0:1, :]),
                        start=False, stop=True)
                copy_rr(fv[mt], pfv)
            for mt in range(NT):
                nc.tensor.matmul(pfs, r(onescol[:, :]), r(fv[mt]),
                                 start=(mt == 0), stop=(mt == NT - 1))
            for h in range(HPG):
                nc.scalar.activation(frep[32 * h:32 * h + 1, :], pfs, AF.Copy)

        # ======== stage C: normalize L/R, var rows ========
        with tc.tile_pool(name="pbc", bufs=2, space="PSUM") as PBC, \
             tc.tile_pool(name="pvr", bufs=2, space="PSUM") as PVR:
            for h in range(HPG):
                hs = slice(32 * h, 32 * h + 1)
                for ncx in range(NC):
                    cs = slice(ncx * 512, (ncx + 1) * 512)
                    # k side: fkn = fk * bcast(1/kn); accumulate fks rows
                    pb = PBC.tile([128, 512], f32, tag="pb", name="pb")
                    nc.tensor.matmul(pb, r(browk[hs, :]), r(kstat[hs, cs]),
                                     start=True, stop=True,
                                     tile_position=(32 * h, 0))
                    nc.vector.tensor_tensor(
                        L[h][64:128, cs], L[h][64:128, cs],
                        pb[64:128, :], ALU.mult)
                    # q side: fqn = fq * bcast(cos_w/qn)
                    pb2 = PBC.tile([128, 512], f32, tag="pb", name="pb2")
                    nc.tensor.matmul(pb2, r(browq[hs, :]), r(qstat[hs, cs]),
                                     start=True, stop=True,
                                     tile_position=(32 * h, 0))
                    nc.vector.tensor_tensor(
                        R[h][64:128, cs], R[h][64:128, cs],
                        pb2[64:128, :], ALU.mult)
            for h in range(HPG):
                nc.vector.reduce_sum(fkst[64:128, h:h + 1], L[h][64:128, :],
                                     axis=AX.X)
            # vr rows: vr = var_w - var_w/(N*cos_w) * (fks . fqn)
            for h in range(HPG):
                hs = slice(32 * h, 32 * h + 1)
                for ncx in range(NC):
                    cs = slice(ncx * 512, (ncx + 1) * 512)
                    pv1 = PVR.tile([1, 512], f32, tag="pvr", name="pv1")
                    nc.tensor.matmul(
                        pv1, r(fkst[64:128, h:h + 1]),
                        r(R[h][64:128, cs]), start=True, stop=True)
                    nc.scalar.activation(
                        vrr[hs, cs], pv1, AF.Identity, bias=vwcol[0:1, :],
                        scale=-(var_w / (N * cos_w)))

        # ======== stage D: scores + out-stage ========
        with tc.tile_pool(name="pss", bufs=3, space="PSUM") as PSS, \
             tc.tile_pool(name="pop", bufs=1, space="PSUM") as POP, \
             tc.tile_pool(name="stp", bufs=4) as STP:
            for h in range(HPG):
                j2, jj = h // 2, h % 2
                hs = slice(32 * h, 32 * h + 1)
                po = [POP.tile([64, 512], f32, tag=f"po{ncx}",
                               name=f"po{h}_{ncx}") for ncx in range(NC)]
                for mt in range(NT):
                    ms = slice(mt * 128, (mt + 1) * 128)
                    pss = PSS.tile([128, 1024], f32, tag="pss", name="pss")
                    nc.tensor.matmul(pss[:, 0:512], r(L[h][:, ms]),
                                     r(R[h][:, 0:512]),
                                     start=True, stop=True)
                    nc.tensor.matmul(pss[:, 512:1024], r(L[h][:, ms]),
                                     r(R[h][:, 512:1024]),
                                     start=True, stop=True)
                    st = STP.tile([128, 1024], f32r, tag="st", name="st")
                    copy_rr(st, pss, seq=(0, 1))
                    for ncx in range(NC):
                        nc.tensor.matmul(
                            po[ncx],
                            r(fv[mt][:, h * 64:(h + 1) * 64]),
                            r(st[:, ncx * 512:(ncx + 1) * 512]),
                            start=(mt == 0), stop=False)
                for ncx in range(NC):
                    cs = slice(ncx * 512, (ncx + 1) * 512)
                    nc.tensor.matmul(
                        po[ncx],
                        r(frep[hs, h * 64:(h + 1) * 64]),
                        r(vrr[hs, cs]), start=False, stop=True,
                        tile_position=(32 * h, 0))
                    copy_rr(oT[j2][jj * 64:(jj + 1) * 64, cs], po[ncx])

        # ======== stage E: W_out projection + store ========
        with tc.tile_pool(name="pe2", bufs=2, space="PSUM") as PE2, \
             tc.tile_pool(name="obp", bufs=3) as OBP:
            for nt in range(NT):
                pf = PE2.tile([128, 512], f32, tag="pf", name="pfe")
                for j2 in range(2):
                    nc.tensor.matmul(
                        pf, r(oT[j2][:, nt * 128:(nt + 1) * 128]),
                        r(wo_sb[:, j2 * 512:(j2 + 1) * 512]),
                        start=(j2 == 0), stop=(j2 == 1))
                obt = OBP.tile([128, DIM], f32, tag="ob", name="obt")
                copy_rr(obt, pf)
                nc.sync.dma_start(
                    out=out_d[:, nt * DIM:(nt + 1) * DIM], in_=obt)

    _lp.__exit__(None, None, None)
    nc.compile()
    return nc


def _host_prep_weights(ln_g, ln_b, W_in, W_out, g):
    """Per-head-group weight layouts (see _build_nc docstring)."""
    W_f = (ln_g[:, None] * W_in)[:, g * IG:(g + 1) * IG]  # [512, 256]
    C = np.eye(DIM_HEAD, dtype=np.float32) - 1.0 / DIM_HEAD

    # k-aug per head: [W_h @ C | W_h] -> [512, 128] each
    wk = np.empty((DIM, HPG * 128), np.float32)
    for h in range(HPG):
        Wh = W_f[:, h * 64:(h + 1) * 64]
        wk[:, h * 128:h * 128 + 64] = Wh @ C
        wk[:, h * 128 + 64:(h + 1) * 128] = Wh
    # c-major SBUF layouts: [p, h*512 + c*128 + i] = wk[c*128+p, h*128+i]
    wk_sb = np.ascontiguousarray(
        wk.reshape(CC, 128, HPG, 128).transpose(1, 2, 0, 3).reshape(128, HPG * DIM))
    wv_sb = np.ascontiguousarray(
        W_f.reshape(CC, 128, IG).transpose(1, 0, 2).reshape(128, CC * IG))
    Wo = W_out[g * IG:(g + 1) * IG, :]  # [256, 512]
    wo_sb = np.ascontiguousarray(
        Wo.reshape(2, 128, DIM).transpose(1, 0, 2).reshape(128, 2 * DIM))

    bW = (ln_b @ W_in)[g * IG:(g + 1) * IG].astype(np.float32)  # [256]
    has_bias = bool(np.any(bW))
    bq = bW[None, :]
    bk = np.empty((1, HPG * 128), np.float32)
    for h in range(HPG):
        bh = bW[h * 64:(h + 1) * 64]
        bk[0, h * 128:h * 128 + 64] = bh @ C
        bk[0, h * 128 + 64:(h + 1) * 128] = bh
    return wk_sb, wv_sb, wo_sb, bq, bk, has_bias


def _prep(q, k, v, ln_g, ln_b, W_in, W_out, b_out, cov_w_raw, var_w_raw):
    q = np.asarray(q, np.float32)
    k = np.asarray(k, np.float32)
    v = np.asarray(v, np.float32)
    ln_g = np.asarray(ln_g, np.float32)
    ln_b = np.asarray(ln_b, np.float32)
    W_in = np.asarray(W_in, np.float32)
    W_out = np.asarray(W_out, np.float32)

    cov_w = float(1.0 / (1.0 + np.exp(-np.float64(cov_w_raw))))
    var_w = float(1.0 / (1.0 + np.exp(-np.float64(var_w_raw))))
    cos_w = 1.0 - cov_w - var_w

    per_g = [_host_prep_weights(ln_g, ln_b, W_in, W_out, g) for g in range(HG)]
    has_bias = any(pg[5] for pg in per_g)
    nc = _build_nc(cos_w, cov_w, var_w, has_bias)

    ident = np.eye(128, dtype=np.float32)
    cst = np.zeros((128, 769), np.float32)
    for h in range(HPG):
        cst[32 * h, 64:128] = cos_w      # browq (mult by cos_w/qn)
        cst[32 * h, 128 + 64:128 + 128] = 1.0  # browk (divide by kn)
    cst[:, 256] = 1.0                    # onescol
    cst[0, 257:769] = 1.0                # ones_row

    def pmaj(x2d):  # [1024, 512] -> [128, 8*512] p-major
        return np.ascontiguousarray(
            x2d.reshape(NT, 128, DIM).transpose(1, 0, 2).reshape(128, NT * DIM))

    in_maps = []
    for core in range(8):
        b, g = core // HG, core % HG
        wk_sb, wv_sb, wo_sb, bq, bk, _ = per_g[g]
        m = {
            "xq": pmaj(q[b]), "xk": pmaj(k[b]), "xv": pmaj(v[b]),
            "wk": wk_sb, "wv": wv_sb, "wo": wo_sb, "ident": ident,
            "cst": cst,
        }
        if has_bias:
            m["bq"] = bq
            m["bk"] = bk
        in_maps.append(m)
    return nc, in_maps


def kernel(q, k, v, ln_g, ln_b, W_in, W_out, b_out, cov_w_raw, var_w_raw):
    from concourse.bass_utils import run_bass_kernel_spmd

    b_out = np.asarray(b_out, np.float32)
    nc, in_maps = _prep(q, k, v, ln_g, ln_b, W_in, W_out, b_out,
                        cov_w_raw, var_w_raw)
    res = run_bass_kernel_spmd(nc, in_maps, list(range(8)))

    def unpmaj(o):  # [128, 8*512] -> [1024, 512]
        return o.reshape(128, NT, DIM).transpose(1, 0, 2).reshape(N, DIM)

    parts = [unpmaj(res.results[c]["out"]) for c in range(8)]
    out = np.stack([parts[2 * b] + parts[2 * b + 1] + b_out
                    for b in range(B)])
    return out.astype(np.float32)


# revision 20
# speedup vs baseline: 1.7083x; 1.0231x over previous
"""Trainium2 Bass kernel for nn_Attention_30562987278646.

Sharding: 8 cores = 4 batches x 2 head-groups (4 heads each).
Per core: LN(q/k/v) -> project -> score matrices -> out = S @ f_v ->
partial @ W_out rows. Host sums the 2 head-group partials per batch.

Key identities / layout:
 - LN applied as one fused activation (x*r + (-mu*r)) per n-tile; ln_g
   folded into W on host.
 - cov term: qc . kc == fq . kc (centering q is free), and kc = C f_k
   with C = I - 1/64 folded into the k-side weights on host. So the
   score matmul is ONE K=128 matmul per (m-tile, n-chunk):
     rows 0:64   kc_h      x  fqc_h (= cov_w/64 * fq)
     rows 64:128 fkn_h     x  fqn_h (= cos_w * fq / qn)
 - var term: relu(1-cos)=1-cos (GAMMA=1, |cos|<=1), mean_m(1-cos) gives
   a per-n row vr; its contribution to the output is rank-1
   (vr[n] * colsum_m(f_v)[e]) and is added in the out-stage PSUM
   accumulation as one K=1 matmul per (head, n-chunk).
"""

import sys
import numpy as np
import ml_dtypes

for _p in ("/opt/trn_rl_repo", "/root/.axon_site/_ro/trn_rl_repo"):
    if _p not in sys.path:
        sys.path.append(_p)

HEADS = 8
DIM_HEAD = 64
LN_EPS = 1e-5
B, N, DIM = 4, 1024, 512
HG = 2                      # head groups (shards along heads)
HPG = HEADS // HG           # heads per group = 4
IG = HPG * DIM_HEAD         # inner dim per group = 256
NT = N // 128               # 8 n-tiles
NC = N // 512               # 2 n-chunks
CC = DIM // 128             # 4 c-chunks


def _build_nc(cos_w: float, cov_w: float, var_w: float, has_bias: bool):
    import concourse.bass as bass
    import concourse.bacc as bacc
    import concourse.tile as tile
    from concourse import mybir

    f32 = mybir.dt.float32
    f32r = mybir.dt.float32r
    bf16 = mybir.dt.bfloat16
    AF = mybir.ActivationFunctionType
    ALU = mybir.AluOpType
    AX = mybir.AxisListType

    def r(ap):
        return ap.bitcast(f32r)

    nc = bacc.Bacc(target_bir_lowering=False, debug=False)
    _lp = nc.allow_low_precision(reason="f32r is 4-byte storage, not low precision")
    _lp.__enter__()

    xin_d = {
        t: nc.declare_dram_parameter(t, [128, NT * DIM], bf16, isOutput=False)
        for t in ("xk", "xq", "xv")
    }
    wk_d = nc.declare_dram_parameter("wk", [128, HPG * DIM], bf16, isOutput=False)
    cst_d = nc.declare_dram_parameter("cst", [128, 769], f32, isOutput=False)
    wv_d = nc.declare_dram_parameter("wv", [128, CC * IG], bf16, isOutput=False)
    wo_d = nc.declare_dram_parameter("wo", [128, 2 * DIM], f32, isOutput=False)
    ident_d = nc.declare_dram_parameter("ident", [128, 128], bf16, isOutput=False)
    if has_bias:
        bq_d = nc.declare_dram_parameter("bq", [1, IG], f32, isOutput=False)
        bk_d = nc.declare_dram_parameter("bk", [1, HPG * 128], f32, isOutput=False)
    out_d = nc.declare_dram_parameter("out", [128, NT * DIM], f32, isOutput=True)

    # engine rotation for bulk copies: scalar (Act) / vector (DVE) / gpsimd
    rot = {"i": 0}

    def copy_rr(dst, src, seq=(0, 1)):
        e = seq[rot["i"] % len(seq)]
        rot["i"] += 1
        if e == 0:
            nc.scalar.activation(dst, src, AF.Copy)
        elif e == 1:
            nc.vector.tensor_copy(dst, src)
        else:
            nc.gpsimd.tensor_copy(dst, src)

    with tile.TileContext(nc) as tc, \
         tc.tile_pool(name="persist", bufs=1) as P:

        # ---- persistent constants ----
        ident_sb = P.tile([128, 128], bf16, name="ident_sb")
        nc.sync.dma_start(out=ident_sb, in_=ident_d[:, :])
        cst = P.tile([128, 769], f32r, name="cst")
        nc.sync.dma_start(out=cst, in_=cst_d[:, :].bitcast(f32r))
        browq = cst[:, 0:128]
        browk = cst[:, 128:256]
        onescol = cst[:, 256:257]
        ones_row = cst[0:1, 257:769]
        eps_sb = P.tile([128, 1], f32, name="eps_sb")
        nc.vector.memset(eps_sb, LN_EPS)
        vwcol = P.tile([97, 1], f32, name="vwcol")
        nc.vector.memset(vwcol, var_w)

        # ---- persistent weights ----
        wv_sb = P.tile([128, CC * IG], bf16, name="wv_sb")
        nc.sync.dma_start(out=wv_sb, in_=wv_d[:, :])
        wo_sb = P.tile([128, 2 * DIM], f32r, name="wo_sb")
        nc.sync.dma_start(out=wo_sb, in_=wo_d[:, :].bitcast(f32r))
        if has_bias:
            bq_sb = P.tile([1, IG], f32r, name="bq_sb")
            nc.sync.dma_start(out=bq_sb, in_=bq_d[:, :].bitcast(f32r))
            bk_sb = P.tile([1, HPG * 128], f32r, name="bk_sb")
            nc.sync.dma_start(out=bk_sb, in_=bk_d[:, :].bitcast(f32r))

        # ---- persistent activations ----
        # L[h]: rows 0:64 kc_h, rows 64:128 fk_h -> fkn_h   [128, N]
        # R[h]: rows 0:64 fqc_h, rows 64:128 fq_h -> fqn_h  [128, N]
        L = [P.tile([128, N], f32r, name=f"L{h}") for h in range(HPG)]
        R = [P.tile([128, N], f32r, name=f"R{h}") for h in range(HPG)]
        fv = [P.tile([128, IG], f32r, name=f"fv{mt}") for mt in range(NT)]
        oT = [P.tile([128, N], f32r, name=f"oT{j}") for j in range(2)]
        # stat rows at partition 32h: qstat = cos_w-ready 1/qn, kstat = 1/kn
        qstat = P.tile([97, N], f32r, name="qstat")
        kstat = P.tile([97, N], f32r, name="kstat")
        vrr = P.tile([97, N], f32r, name="vrr")
        fkst = P.tile([128, HPG], f32r, name="fkst")
        frep = P.tile([97, IG], f32r, name="frep")

        zst = P.tile([97, N], f32, name="zst")
        nc.vector.memset(zst, 1.0)
        nc.scalar.activation(qstat, zst, AF.Copy)
        nc.scalar.activation(kstat, zst, AF.Copy)

        # ======== stages A+B: load, LN, transpose, project ========
        with tc.tile_pool(name="xa", bufs=1) as XA, \
             tc.tile_pool(name="zt", bufs=4) as ZT, \
             tc.tile_pool(name="sqp", bufs=4) as SQP, \
             tc.tile_pool(name="smal", bufs=2) as SM, \
             tc.tile_pool(name="wkp", bufs=1) as WKP, \
             tc.tile_pool(name="pt", bufs=2, space="PSUM") as PT, \
             tc.tile_pool(name="pb", bufs=2, space="PSUM") as PB, \
             tc.tile_pool(name="pstat", bufs=2, space="PSUM") as PST:

            wk_sb = WKP.tile([128, HPG * DIM], bf16, name="wk_sb")
            nc.sync.dma_start(out=wk_sb, in_=wk_d[:, :])

            xin = {}
            xT = {}
            for t in ("xk", "xq", "xv"):
                xin[t] = XA.tile([128, NT * DIM], bf16, tag=f"xin{t}",
                                 name=f"xin{t}")
                for hf in range(2):
                    cs = slice(hf * (NT * DIM // 2), (hf + 1) * (NT * DIM // 2))
                    nc.sync.dma_start(out=xin[t][:, cs], in_=xin_d[t][:, cs])
                xT[t] = XA.tile([128, CC * N], bf16, tag=f"xT{t}",
                                name=f"xT{t}")

            # --- stage A per tensor: LN stats (batched), fused LN, transpose
            for t in ("xk", "xq", "xv"):
                mvt = SM.tile([128, 2 * NT], f32, tag="mvt", name="mvt")
                stt = SQP.tile([128, nc.vector.BN_STATS_DIM], f32, tag="bns", name="stt")
                for nt in range(NT):
                    nc.vector.bn_stats(
                        out=stt, in_=xin[t][:, nt * DIM:(nt + 1) * DIM])
                    nc.vector.bn_aggr(out=mvt[:, 2 * nt:2 * nt + 2], in_=stt)
                mv3 = mvt.rearrange("p (t s) -> p t s", s=2)
                rin = SM.tile([128, NT], f32, tag="rin", name="rin")
                nc.scalar.activation(rin, mv3[:, :, 1:2], AF.Sqrt, bias=eps_sb)
                nc.vector.reciprocal(rin, rin)
                nmr = SM.tile([128, NT], f32, tag="nmr", name="nmr")
                nc.vector.tensor_tensor(nmr, mv3[:, :, 0:1], rin, ALU.mult)
                nc.vector.tensor_scalar_mul(nmr, nmr, -1.0)
                xT3 = xT[t].rearrange("p (c n) -> p c n", c=CC)
                for nt in range(NT):
                    zt = ZT.tile([128, DIM], bf16, tag="zt", name="zt")
                    nc.gpsimd.tensor_scalar(
                        zt, xin[t][:, nt * DIM:(nt + 1) * DIM],
                        rin[:, nt:nt + 1], nmr[:, nt:nt + 1],
                        ALU.mult, ALU.add)
                    pt = PT.tile([128, 512], bf16, tag="pt", name="pt")
                    for c in range(CC):
                        nc.tensor.transpose(
                            pt[:, c * 128:(c + 1) * 128],
                            zt[:, c * 128:(c + 1) * 128], ident_sb)
                    nc.vector.tensor_copy(
                        xT3[:, :, nt * 128:(nt + 1) * 128], pt)

            # --- stage B-k: aug projection -> L, stats (kstat rows hold kn)
            for h in range(HPG):
                for ncx in range(NC):
                    cs = slice(ncx * 512, (ncx + 1) * 512)
                    pf = PB.tile([128, 512], f32, tag="pf", name="pf")
                    for c in range(CC):
                        nc.tensor.matmul(
                            pf, wk_sb[:, h * DIM + c * 128:h * DIM + (c + 1) * 128],
                            xT["xk"][:, c * N + ncx * 512:c * N + (ncx + 1) * 512],
                            start=(c == 0), stop=(c == 3 and not has_bias))
                    if has_bias:
                        nc.tensor.matmul(
                            pf, r(bk_sb[0:1, h * 128:(h + 1) * 128]),
                            r(ones_row[0:1, :]), start=False, stop=True)
                    copy_rr(L[h][:, cs], pf)
                    sq = SQP.tile([64, 512], f32r, tag="sq", name="sq")
                    nc.gpsimd.tensor_tensor(sq, L[h][64:128, cs],
                                            L[h][64:128, cs], ALU.mult)
                    ps1 = PST.tile([1, 512], f32, tag="ps", name="ps1")
                    nc.tensor.matmul(ps1, r(onescol[0:64, :]), r(sq),
                                     start=True, stop=True)
                    nc.scalar.activation(kstat[32 * h:32 * h + 1, cs], ps1,
                                         AF.Sqrt)
                if h == 1:
                    nc.vector.reciprocal(kstat[0:33, :], kstat[0:33, :])
            nc.vector.reciprocal(kstat[64:97, :], kstat[64:97, :])

            # --- stage B-q: plain projection (shared weights wv) -> R, stats
            for hp in range(2):
                for ncx in range(NC):
                    cs = slice(ncx * 512, (ncx + 1) * 512)
                    pf = PB.tile([128, 512], f32, tag="pf", name="pf")
                    for c in range(CC):
                        nc.tensor.matmul(
                            pf, wv_sb[:, c * IG + hp * 128:c * IG + (hp + 1) * 128],
                            xT["xq"][:, c * N + ncx * 512:c * N + (ncx + 1) * 512],
                            start=(c == 0), stop=(c == 3 and not has_bias))
                    if has_bias:
                        nc.tensor.matmul(
                            pf, r(bq_sb[0:1, hp * 128:(hp + 1) * 128]),
                            r(ones_row[0:1, :]), start=False, stop=True)
                    for j in range(2):
                        h = 2 * hp + j
                        fq = pf[j * 64:(j + 1) * 64, :]
                        if j == 0:
                            nc.vector.tensor_scalar_mul(
                                R[h][0:64, cs], fq, cov_w / DIM_HEAD)
                        else:
                            nc.scalar.mul(R[h][0:64, cs], fq,
                                          cov_w / DIM_HEAD)
                        copy_rr(R[h][64:128, cs], fq, seq=(0, 1))
                        sq = SQP.tile([64, 512], f32r, tag="sq", name="sq")
                        nc.gpsimd.tensor_tensor(sq, R[h][64:128, cs],
                                                R[h][64:128, cs], ALU.mult)
                        ps1 = PST.tile([1, 512], f32, tag="ps", name="ps1")
                        nc.tensor.matmul(ps1, r(onescol[0:64, :]), r(sq),
                                         start=True, stop=True)
                        nc.scalar.activation(qstat[32 * h:32 * h + 1, cs],
                                             ps1, AF.Sqrt)
                if hp == 0:
                    nc.vector.reciprocal(qstat[0:33, :], qstat[0:33, :])
            nc.vector.reciprocal(qstat[64:97, :], qstat[64:97, :])

            # --- stage B-v: projection -> fv (n-major), fvsum
            pfs = PST.tile([1, IG], f32, tag="pfs", name="pfs")
            for mt in range(NT):
                pfv = PB.tile([128, 512], f32, tag="pf", name="pfv")[:, 0:IG]
                for c in range(CC):
                    nc.tensor.matmul(
                        pfv, xT["xv"][:, c * N + mt * 128:c * N + (mt + 1) * 128],
                        wv_sb[:, c * IG:(c + 1) * IG],
                        start=(c == 0), stop=(c == 3 and not has_bias))
                if has_bias:
                    nc.tensor.matmul(
                        pfv, r(ones_row[0:1, 0:128]), r(bq_sb[0:1, :]),
                        start=False, stop=True)
                copy_rr(fv[mt], pfv)
            for mt in range(NT):
                nc.tensor.matmul(pfs, r(onescol[:, :]), r(fv[mt]),
                                 start=(mt == 0), stop=(mt == NT - 1))
            for h in range(HPG):
                nc.scalar.activation(frep[32 * h:32 * h + 1, :], pfs, AF.Copy)

        # ======== stage C: normalize L/R, var rows ========
        with tc.tile_pool(name="pbc", bufs=2, space="PSUM") as PBC, \
             tc.tile_pool(name="pvr", bufs=2, space="PSUM") as PVR:
            for h in range(HPG):
                hs = slice(32 * h, 32 * h + 1)
                for ncx in range(NC):
                    cs = slice(ncx * 512, (ncx + 1) * 512)
                    # k side: fkn = fk * bcast(1/kn); q: fqn *= bcast(1/qn)
                    pb = PBC.tile([128, 512], f32, tag="pb", name="pb")
                    nc.tensor.matmul(pb, r(browk[hs, :]), r(kstat[hs, cs]),
                                     start=True, stop=True,
                                     tile_position=(32 * h, 0))
                    nc.vector.tensor_tensor(
                        L[h][64:128, cs], L[h][64:128, cs],
                        pb[64:128, :], ALU.mult)
                    pb2 = PBC.tile([128, 512], f32, tag="pb", name="pb2")
                    nc.tensor.matmul(pb2, r(browq[hs, :]), r(qstat[hs, cs]),
                                     start=True, stop=True,
                                     tile_position=(32 * h, 0))
                    nc.vector.tensor_tensor(
                        R[h][64:128, cs], R[h][64:128, cs],
                        pb2[64:128, :], ALU.mult)
            for h in range(HPG):
                nc.vector.reduce_sum(fkst[64:128, h:h + 1], L[h][64:128, :],
                                     axis=AX.X)
            # vr rows: vr = var_w - var_w/(N*cos_w) * (fks . fqn)
            for h in range(HPG):
                hs = slice(32 * h, 32 * h + 1)
                for ncx in range(NC):
                    cs = slice(ncx * 512, (ncx + 1) * 512)
                    pv1 = PVR.tile([1, 512], f32, tag="pvr", name="pv1")
                    nc.tensor.matmul(
                        pv1, r(fkst[64:128, h:h + 1]),
                        r(R[h][64:128, cs]), start=True, stop=True)
                    nc.scalar.activation(
                        vrr[hs, cs], pv1, AF.Identity, bias=vwcol[0:1, :],
                        scale=-(var_w / (N * cos_w)))

        # ======== stage D: scores + out-stage ========
        with tc.tile_pool(name="pss", bufs=3, space="PSUM") as PSS, \
             tc.tile_pool(name="pop", bufs=1, space="PSUM") as POP, \
             tc.tile_pool(name="stp", bufs=4) as STP:
            for h in range(HPG):
                j2, jj = h // 2, h % 2
                hs = slice(32 * h, 32 * h + 1)
                po = [POP.tile([64, 512], f32, tag=f"po{ncx}",
                               name=f"po{h}_{ncx}") for ncx in range(NC)]
                for mt in range(NT):
                    ms = slice(mt * 128, (mt + 1) * 128)
                    pss = PSS.tile([128, 1024], f32, tag="pss", name="pss")
                    nc.tensor.matmul(pss[:, 0:512], r(L[h][:, ms]),
                                     r(R[h][:, 0:512]),
                                     start=True, stop=True)
                    nc.tensor.matmul(pss[:, 512:1024], r(L[h][:, ms]),
                                     r(R[h][:, 512:1024]),
                                     start=True, stop=True)
                    st = STP.tile([128, 1024], f32r, tag="st", name="st")
                    copy_rr(st, pss, seq=(0, 1))
                    for ncx in range(NC):
                        nc.tensor.matmul(
                            po[ncx],
                            r(fv[mt][:, h * 64:(h + 1) * 64]),
                            r(st[:, ncx * 512:(ncx + 1) * 512]),
                            start=(mt == 0), stop=False)
                for ncx in range(NC):
                    cs = slice(ncx * 512, (ncx + 1) * 512)
                    nc.tensor.matmul(
                        po[ncx],
                        r(frep[hs, h * 64:(h + 1) * 64]),
                        r(vrr[hs, cs]), start=False, stop=True,
                        tile_position=(32 * h, 0))
                    copy_rr(oT[j2][jj * 64:(jj + 1) * 64, cs], po[ncx])

        # ======== stage E: W_out projection + store ========
        with tc.tile_pool(name="pe2", bufs=2, space="PSUM") as PE2, \
             tc.tile_pool(name="obp", bufs=3) as OBP:
            for nt in range(NT):
                pf = PE2.tile([128, 512], f32, tag="pf", name="pfe")
                for j2 in range(2):
                    nc.tensor.matmul(
                        pf, r(oT[j2][:, nt * 128:(nt + 1) * 128]),
                        r(wo_sb[:, j2 * 512:(j2 + 1) * 512]),
                        start=(j2 == 0), stop=(j2 == 1))
                obt = OBP.tile([128, DIM], f32, tag="ob", name="obt")
                copy_rr(obt, pf)
                nc.sync.dma_start(
                    out=out_d[:, nt * DIM:(nt + 1) * DIM], in_=obt)

    _lp.__exit__(None, None, None)
    nc.compile()
    return nc


def _host_prep_weights(ln_g, ln_b, W_in, W_out, g):
    """Per-head-group weight layouts (see _build_nc docstring)."""
    W_f = (ln_g[:, None] * W_in)[:, g * IG:(g + 1) * IG]  # [512, 256]
    C = np.eye(DIM_HEAD, dtype=np.float32) - 1.0 / DIM_HEAD

    # k-aug per head: [W_h @ C | W_h] -> [512, 128] each
    wk = np.empty((DIM, HPG * 128), np.float32)
    for h in range(HPG):
        Wh = W_f[:, h * 64:(h + 1) * 64]
        wk[:, h * 128:h * 128 + 64] = Wh @ C
        wk[:, h * 128 + 64:(h + 1) * 128] = Wh
    # c-major SBUF layouts: [p, h*512 + c*128 + i] = wk[c*128+p, h*128+i]
    wk_sb = np.ascontiguousarray(
        wk.reshape(CC, 128, HPG, 128).transpose(1, 2, 0, 3).reshape(128, HPG * DIM))
    wv_sb = np.ascontiguousarray(
        W_f.reshape(CC, 128, IG).transpose(1, 0, 2).reshape(128, CC * IG))
    Wo = W_out[g * IG:(g + 1) * IG, :]  # [256, 512]
    wo_sb = np.ascontiguousarray(
        Wo.reshape(2, 128, DIM).transpose(1, 0, 2).reshape(128, 2 * DIM))

    bW = (ln_b @ W_in)[g * IG:(g + 1) * IG].astype(np.float32)  # [256]
    has_bias = bool(np.any(bW))
    bq = bW[None, :]
    bk = np.empty((1, HPG * 128), np.float32)
    for h in range(HPG):
        bh = bW[h * 64:(h + 1) * 64]
        bk[0, h * 128:h * 128 + 64] = bh @ C
        bk[0, h * 128 + 64:(h + 1) * 128] = bh
    return wk_sb, wv_sb, wo_sb, bq, bk, has_bias


def _prep(q, k, v, ln_g, ln_b, W_in, W_out, b_out, cov_w_raw, var_w_raw):
    q = np.asarray(q, np.float32)
    k = np.asarray(k, np.float32)
    v = np.asarray(v, np.float32)
    ln_g = np.asarray(ln_g, np.float32)
    ln_b = np.asarray(ln_b, np.float32)
    W_in = np.asarray(W_in, np.float32)
    W_out = np.asarray(W_out, np.float32)

    cov_w = float(1.0 / (1.0 + np.exp(-np.float64(cov_w_raw))))
    var_w = float(1.0 / (1.0 + np.exp(-np.float64(var_w_raw))))
    cos_w = 1.0 - cov_w - var_w

    per_g = [_host_prep_weights(ln_g, ln_b, W_in, W_out, g) for g in range(HG)]
    has_bias = any(pg[5] for pg in per_g)
    nc = _build_nc(cos_w, cov_w, var_w, has_bias)

    ident = np.eye(128, dtype=np.float32).astype(ml_dtypes.bfloat16)
    cst = np.zeros((128, 769), np.float32)
    for h in range(HPG):
        cst[32 * h, 64:128] = cos_w      # browq (mult by cos_w/qn)
        cst[32 * h, 128 + 64:128 + 128] = 1.0  # browk (divide by kn)
    cst[:, 256] = 1.0                    # onescol
    cst[0, 257:769] = 1.0                # ones_row

    def pmaj(x2d):  # [1024, 512] -> [128, 8*512] p-major, bf16
        return np.ascontiguousarray(
            x2d.reshape(NT, 128, DIM).transpose(1, 0, 2).reshape(
                128, NT * DIM).astype(ml_dtypes.bfloat16))

    in_maps = []
    for core in range(8):
        b, g = core // HG, core % HG
        wk_sb, wv_sb, wo_sb, bq, bk, _ = per_g[g]
        m = {
            "xq": pmaj(q[b]), "xk": pmaj(k[b]), "xv": pmaj(v[b]),
            "wk": wk_sb.astype(ml_dtypes.bfloat16),
            "wv": wv_sb.astype(ml_dtypes.bfloat16),
            "wo": wo_sb, "ident": ident,
            "cst": cst,
        }
        if has_bias:
            m["bq"] = bq
            m["bk"] = bk
        in_maps.append(m)
    return nc, in_maps


def kernel(q, k, v, ln_g, ln_b, W_in, W_out, b_out, cov_w_raw, var_w_raw):
    from concourse.bass_utils import run_bass_kernel_spmd

    b_out = np.asarray(b_out, np.float32)
    nc, in_maps = _prep(q, k, v, ln_g, ln_b, W_in, W_out, b_out,
                        cov_w_raw, var_w_raw)
    res = run_bass_kernel_spmd(nc, in_maps, list(range(8)))

    def unpmaj(o):  # [128, 8*512] -> [1024, 512]
        return o.reshape(128, NT, DIM).transpose(1, 0, 2).reshape(N, DIM)

    parts = [unpmaj(res.results[c]["out"]) for c in range(8)]
    out = np.stack([parts[2 * b] + parts[2 * b + 1] + b_out
                    for b in range(B)])
    return out.astype(np.float32)
